# revision 2
# baseline (speedup 1.0000x reference)
"""Trainium2 Bass kernel for nn_CenterCrop: per-sample resize(short-side=256)
+ center-crop(224), bilinear, batch sharded over 8 NeuronCores.

Bilinear resize is separable: out = S^T @ img @ G with per-sample sparse
interpolation matrices S (vertical) and G (horizontal), built on the host
from the h/w metadata. The gather+lerp runs on the PE array as matmuls:
  pass1: tmp1_T[x, j] = sum_y img[y, x] * S[y, j]   (img tiles stationary)
  pass2: out[j, i]    = sum_x tmp1_T[x, j] * G[x, i] (tmp1 tiles stationary)

Perf structure (v2, from the 44-51us baseline):
- fp16 data path end-to-end (bilinear abs-rel error ~8e-4, gate 2e-2).
- The kernel is DMA-byte-bound (~360 GB/s/core aggregate over 16 queues),
  so the contraction tiles are BALANCED instead of 128-padded: y-tiles of
  R=ceil(wh/n_yt) rows and x-chunks of X=ceil(ww/n_xt) cols. No zero rows
  are shipped; matmuls contract [0:rows_t] partitions only. This cuts
  ~18% of input traffic vs 128-row tiles.
- One DMA per slot: the image window (partition-major) and the packed S|G
  band columns ship as a single [P, F] blob, so each dma_start's ~600ns
  sequencer setup (DMA_SEQ_TIME) is paid 8x for inputs, not 19x, and the
  first compute gates on one semaphore.
- PSUM: four dedicated double-buffered pools (ps1a/ps1b/ps2a/ps2b = 8
  banks exactly) so a drain never blocks the next accumulation session.
- PSUM channel pairing: c0+c1 share a [*,448] tile and one accumulation
  session (has_written bits make the split ranges exact); drains
  ping-pong between Act and DVE so neither sees the full burst rate.
- Outputs DMA per (slot, jb) half so the tail after the last matmul is
  one 112-row drain + a ~300KB DMA.
- Software pipelining: pass2 of slot s-1 is emitted after pass1 of slot
  s; samples are sorted by min(h,w) and dealt round-robin so slot s on
  every core holds same-sized windows (SPMD: one program, 8 cores).
"""

import sys
import os

for _p in ("/opt/trn_rl_repo",):
    if os.path.isdir(_p) and _p not in sys.path:
        sys.path.insert(0, _p)

import numpy as np

OUT_H = 224
OUT_W = 224
RESIZE_TO = np.float32(256.0)
B_FULL = 64
N_CORES = 8
B_LOC = B_FULL // N_CORES  # 8 slots per core
C = 3
H = 512
W = 512  # image width after stripping the metadata column (stored width 513)

LAST_EXEC_NS = None
LAST_RESULTS = None
_NC_CACHE = {}

USE_F16 = os.environ.get("CENTERCROP_F16", "1") == "1"


def _interp_matrices(h, w):
    """Full S [512, OUT_H], G [512, OUT_W] fp32 interpolation matrices,
    mirroring the reference fp32 math bit-for-bit."""
    f32 = np.float32
    h = f32(h)
    w = f32(w)
    min_dim = min(h, w)
    scale = RESIZE_TO / min_dim
    h_res = np.round(h * scale)
    w_res = np.round(w * scale)
    top = np.round((h_res - f32(OUT_H)) / f32(2.0))
    left = np.round((w_res - f32(OUT_W)) / f32(2.0))

    def axis_mat(n_out, offset, dim, dim_res, n_src):
        idx = np.arange(n_out, dtype=np.float32) + offset
        src = np.clip((idx + f32(0.5)) * dim / dim_res - f32(0.5),
                      f32(0.0), dim - f32(1.0))
        p0f = np.floor(src)
        frac = src - p0f
        imax = np.int32(dim) - 1
        p0 = np.clip(p0f.astype(np.int32), 0, imax)
        p1 = np.minimum(p0 + 1, imax)
        mat = np.zeros((n_src, n_out), np.float32)
        cols = np.arange(n_out)
        np.add.at(mat, (p0, cols), f32(1.0) - frac)
        np.add.at(mat, (p1, cols), frac)
        return mat

    S = axis_mat(OUT_H, top, h, h_res, H)
    G = axis_mat(OUT_W, left, w, w_res, W)
    return S, G


def _bands(mat, tile_rows, n_tiles):
    """Per-tile [lo, hi) columns with any nonzero; None if empty."""
    out = []
    for t in range(n_tiles):
        rows = mat[t * tile_rows:(t + 1) * tile_rows]
        nz = np.nonzero(rows.any(axis=0))[0]
        out.append(None if nz.size == 0 else (int(nz[0]), int(nz[-1]) + 1))
    return out


def _union_bands(band_lists):
    n = len(band_lists[0])
    out = []
    for t in range(n):
        los = [b[t][0] for b in band_lists if b[t] is not None]
        his = [b[t][1] for b in band_lists if b[t] is not None]
        out.append(None if not los else (min(los), max(his)))
    return out


def _band_offsets(bands):
    """Packed running offsets for non-empty bands; total width last."""
    offs = []
    off = 0
    for b in bands:
        if b is None:
            offs.append(None)
        else:
            offs.append(off)
            off += b[1] - b[0]
    return offs, off


def _prepare(x):
    """Host prep: per-sample windows/matrices, sorted slot assignment,
    per-core packed blobs, and the per-slot program parameters."""
    dtd_np = np.float16 if USE_F16 else np.float32
    h_all = x[:, 0, 0, -1].astype(np.float32)
    w_all = x[:, 1, 0, -1].astype(np.float32)

    samples = []
    for b in range(B_FULL):
        S, G = _interp_matrices(h_all[b], w_all[b])
        ynz = np.nonzero(S.any(axis=1))[0]
        xnz = np.nonzero(G.any(axis=1))[0]
        y0, y1 = int(ynz[0]), int(ynz[-1]) + 1
        x0, x1 = int(xnz[0]), int(xnz[-1]) + 1
        samples.append(dict(S=S[y0:y1], G=G[x0:x1], y0=y0, x0=x0,
                            wh=y1 - y0, ww=x1 - x0))

    order = np.argsort(np.minimum(h_all, w_all), kind="stable")
    # slot s, core c -> sample order[s*N_CORES + c]; slot index ascending
    # in window size.
    assign = [[int(order[s * N_CORES + c]) for c in range(N_CORES)]
              for s in range(B_LOC)]

    slot_params = []
    in_maps = [{} for _ in range(N_CORES)]
    for s in range(B_LOC):
        sids = assign[s]
        wh = max(samples[i]["wh"] for i in sids)
        ww = max(samples[i]["ww"] for i in sids)
        n_yt = (wh + 127) // 128
        n_xt = (ww + 127) // 128
        R = (wh + n_yt - 1) // n_yt   # balanced y-tile rows
        X = (ww + n_xt - 1) // n_xt   # balanced x-chunk cols
        P = max(R, X)

        sb_list, gb_list, mats = [], [], []
        for i in sids:
            sp = samples[i]
            Sw = np.zeros((n_yt * R, OUT_H), np.float32)
            Sw[:sp["wh"]] = sp["S"]
            Gw = np.zeros((n_xt * X, OUT_W), np.float32)
            Gw[:sp["ww"]] = sp["G"]
            sb_list.append(_bands(Sw, R, n_yt))
            gb_list.append(_bands(Gw, X, n_xt))
            mats.append((Sw, Gw))
        sbands = tuple(_union_bands(sb_list))
        gbands = tuple(_union_bands(gb_list))
        s_offs, s_tot = _band_offsets(sbands)
        g_offs, g_tot = _band_offsets(gbands)
        img_w = C * n_yt * ww
        F = img_w + s_tot + g_tot
        slot_params.append((n_yt, n_xt, wh, ww, R, X, P,
                            sbands, gbands))

        for ci in range(N_CORES):
            sp = samples[sids[ci]]
            Sw, Gw = mats[ci]
            blob = np.zeros((P, F), dtd_np)
            win = x[sids[ci], :, sp["y0"]:sp["y0"] + sp["wh"],
                    sp["x0"]:sp["x0"] + sp["ww"]]
            for t in range(n_yt):
                rows = win[:, t * R:(t + 1) * R]  # [C, rt, ww_s]
                rt = rows.shape[1]
                for cch in range(C):
                    base = cch * n_yt * ww + t * ww
                    blob[:rt, base:base + sp["ww"]] = rows[cch]
            St = Sw.reshape(n_yt, R, OUT_H)
            Gt = Gw.reshape(n_xt, X, OUT_W)
            for t in range(n_yt):
                if sbands[t] is not None:
                    lo, hi = sbands[t]
                    o = img_w + s_offs[t]
                    blob[:R, o:o + hi - lo] = St[t, :, lo:hi]
            for xb in range(n_xt):
                if gbands[xb] is not None:
                    lo, hi = gbands[xb]
                    o = img_w + s_tot + g_offs[xb]
                    blob[:X, o:o + hi - lo] = Gt[xb, :, lo:hi]
            in_maps[ci][f"blob{s}"] = blob
    return tuple(slot_params), in_maps, assign


def _build_nc(slot_params):
    import concourse.bacc as bacc
    import concourse.mybir as mybir
    import concourse.tile as tile

    dt = mybir.dt.float32
    dtd = mybir.dt.float16 if USE_F16 else mybir.dt.float32
    nc = bacc.Bacc(
        "TRN2",
        target_bir_lowering=False,
        debug=False,
        enable_asserts=False,
        num_devices=N_CORES,
    )
    n_slots = len(slot_params)
    meta = []
    blob_in = []
    for s, (n_yt, n_xt, wh, ww, R, X, P, sbands, gbands) in \
            enumerate(slot_params):
        s_offs, s_tot = _band_offsets(sbands)
        g_offs, g_tot = _band_offsets(gbands)
        img_w = C * n_yt * ww
        F = img_w + s_tot + g_tot
        meta.append((s_offs, g_offs, img_w, img_w + s_tot, F))
        blob_in.append(nc.dram_tensor(f"blob{s}", [P, F], dtd,
                                      kind="ExternalInput"))
    out = nc.dram_tensor("out", [B_LOC, 2, 112, C, OUT_W], dtd,
                         kind="ExternalOutput")

    # head waits on slot_order[0]'s DMA; the tail drain is slot_order[-1]'s
    # pass2 + output DMA (keep both ends small)
    slot_order = list(range(1, n_slots)) + [0]

    with tile.TileContext(nc) as tc:
        with (
            tc.tile_pool(name="blob", bufs=n_slots) as blob_pool,
            tc.tile_pool(name="tmp", bufs=3) as tmp_pool,
            tc.tile_pool(name="outp", bufs=4) as out_pool,
            tc.tile_pool(name="ps1a", bufs=2, space="PSUM") as ps1a_pool,
            tc.tile_pool(name="ps1b", bufs=2, space="PSUM") as ps1b_pool,
            tc.tile_pool(name="ps2a", bufs=2, space="PSUM") as ps2a_pool,
            tc.tile_pool(name="ps2b", bufs=2, space="PSUM") as ps2b_pool,
        ):
            blob_tiles = [None] * n_slots
            tmp_tiles = [None] * n_slots

            # all input DMAs issued up front in compute order
            for s in slot_order:
                n_yt, n_xt, wh, ww, R, X, P, sbands, gbands = slot_params[s]
                F = meta[s][4]
                b_sb = blob_pool.tile([P, F], dtd)
                nc.sync.dma_start(b_sb[:], blob_in[s][:])
                blob_tiles[s] = b_sb

            def pass1(s):
                n_yt, n_xt, wh, ww, R, X, P, sbands, gbands = slot_params[s]
                s_offs, g_offs, img_w, g_base, F = meta[s]
                b_sb = blob_tiles[s]
                s_emit = [t for t in range(n_yt) if sbands[t] is not None]
                tmp_sb = tmp_pool.tile([128, n_xt, C, OUT_H], dtd)
                tmp_tiles[s] = tmp_sb
                for xb in range(n_xt):
                    xlo = xb * X
                    xn = min(X, ww - xlo)
                    ps1a = ps1a_pool.tile([128, 2 * OUT_H], dt, tag='ps1a')
                    ps1b = ps1b_pool.tile([128, OUT_H], dt, tag='ps1b')
                    # c0+c1 share one accumulation session in ps1a: PSUM
                    # has_written bits make the split ranges exact
                    for c in range(C):
                        base = c * OUT_H if c < 2 else 0
                        ps = ps1a if c < 2 else ps1b
                        for i_t, t in enumerate(s_emit):
                            lo, hi = sbands[t]
                            rt = min(R, wh - t * R)
                            ib = c * n_yt * ww + t * ww + xlo
                            o = img_w + s_offs[t]
                            first = i_t == 0 and (c == 0 or c == 2)
                            last = (i_t == len(s_emit) - 1
                                    and (c == 1 or c == 2))
                            nc.tensor.matmul(
                                ps[:xn, base + lo:base + hi],
                                b_sb[:rt, ib:ib + xn],
                                b_sb[:rt, o:o + hi - lo],
                                start=first,
                                stop=last,
                                skip_group_check=True,
                            )
                    # ping-pong the drains so each engine sees half the
                    # burst rate
                    if xb % 2 == 0:
                        nc.scalar.copy(tmp_sb[:xn, xb, 0:2, :],
                                       ps1a[:xn, :])
                        nc.vector.tensor_copy(tmp_sb[:xn, xb, 2, :],
                                              ps1b[:xn, :OUT_H])
                    else:
                        nc.vector.tensor_copy(tmp_sb[:xn, xb, 0:2, :],
                                              ps1a[:xn, :])
                        nc.scalar.copy(tmp_sb[:xn, xb, 2, :],
                                       ps1b[:xn, :OUT_H])

            def pass2(s):
                n_yt, n_xt, wh, ww, R, X, P, sbands, gbands = slot_params[s]
                s_offs, g_offs, img_w, g_base, F = meta[s]
                b_sb = blob_tiles[s]
                tmp_sb = tmp_tiles[s]
                g_emit = [t for t in range(n_xt) if gbands[t] is not None]
                for jb in range(2):
                    out_sb = out_pool.tile([112, C, OUT_W], dtd, tag="out")
                    ps2a = ps2a_pool.tile([112, 2 * OUT_W], dt, tag='ps2a')
                    ps2b = ps2b_pool.tile([112, OUT_W], dt, tag='ps2b')
                    for c in range(C):
                        base = c * OUT_W if c < 2 else 0
                        ps = ps2a if c < 2 else ps2b
                        for i_t, xb in enumerate(g_emit):
                            lo, hi = gbands[xb]
                            o = g_base + g_offs[xb]
                            xn = min(X, ww - xb * X)
                            first = i_t == 0 and (c == 0 or c == 2)
                            last = (i_t == len(g_emit) - 1
                                    and (c == 1 or c == 2))
                            nc.tensor.matmul(
                                ps[:, base + lo:base + hi],
                                tmp_sb[:xn, xb, c,
                                       jb * 112:(jb + 1) * 112],
                                b_sb[:xn, o:o + hi - lo],
                                start=first,
                                stop=last,
                                skip_group_check=True,
                            )
                    if jb == 0:
                        nc.vector.tensor_copy(out_sb[:, 0:2, :],
                                              ps2a[:, :])
                        nc.scalar.copy(out_sb[:, 2, :],
                                       ps2b[:, :OUT_W])
                    else:
                        nc.scalar.copy(out_sb[:, 0:2, :],
                                       ps2a[:, :])
                        nc.vector.tensor_copy(out_sb[:, 2, :],
                                              ps2b[:, :OUT_W])
                    nc.sync.dma_start(out[s, jb], out_sb[:])

            # software pipeline: pass2 of the previous slot is emitted after
            # pass1 of the current one, so PE never head-of-line blocks on
            # the ps1->tmp drains.
            prev = None
            for s in slot_order:
                pass1(s)
                if prev is not None:
                    pass2(prev)
                prev = s
            pass2(prev)
    nc.compile()
    return nc


def kernel(x, _trace=False):
    global LAST_EXEC_NS, LAST_RESULTS
    from concourse.bass_utils import run_bass_kernel_spmd

    x = np.ascontiguousarray(np.asarray(x), dtype=np.float32)
    assert x.shape == (B_FULL, C, H, W + 1), x.shape

    slot_params, in_maps, assign = _prepare(x)
    key = (slot_params, USE_F16)
    if key not in _NC_CACHE:
        _NC_CACHE[key] = _build_nc(slot_params)
    nc = _NC_CACHE[key]

    res = run_bass_kernel_spmd(nc, in_maps, list(range(N_CORES)), trace=_trace)
    LAST_EXEC_NS = res.exec_time_ns
    LAST_RESULTS = res

    out_full = np.empty((B_FULL, C, OUT_H, OUT_W), np.float32)
    for s in range(B_LOC):
        for c in range(N_CORES):
            # device layout [2, 112, C, OUT_W]; j = jb*112 + p
            arr = res.results[c]["out"][s].astype(np.float32)
            out_full[assign[s][c]] = arr.transpose(2, 0, 1, 3).reshape(
                C, OUT_H, OUT_W)
    return out_full


# revision 3
# speedup vs baseline: 1.8837x; 1.8837x over previous
"""Trainium2 Bass kernel for nn_CenterCrop: per-sample resize(short-side=256)
+ center-crop(224), bilinear, batch sharded over 8 NeuronCores.

Bilinear resize is separable: out = S^T @ img @ G with per-sample sparse
interpolation matrices S (vertical) and G (horizontal), built on the host
from the h/w metadata. The gather+lerp runs on the PE array as matmuls:
  pass1: tmp1_T[x, j] = sum_y img[y, x] * S[y, j]   (img tiles stationary)
  pass2: out[j, i]    = sum_x tmp1_T[x, j] * G[x, i] (tmp1 tiles stationary)

Perf structure (v2, from the 44-51us baseline):
- fp16 data path end-to-end (bilinear abs-rel error ~8e-4, gate 2e-2).
- The kernel is DMA-byte-bound (~360 GB/s/core aggregate over 16 queues),
  so the contraction tiles are BALANCED instead of 128-padded: y-tiles of
  R=ceil(wh/n_yt) rows and x-chunks of X=ceil(ww/n_xt) cols. No zero rows
  are shipped; matmuls contract [0:rows_t] partitions only. This cuts
  ~18% of input traffic vs 128-row tiles.
- One DMA per slot: the image window (partition-major) and the packed S|G
  band columns ship as a single [P, F] blob, so each dma_start's ~600ns
  sequencer setup (DMA_SEQ_TIME) is paid 8x for inputs, not 19x, and the
  first compute gates on one semaphore.
- PSUM: four dedicated double-buffered pools (ps1a/ps1b/ps2a/ps2b = 8
  banks exactly) so a drain never blocks the next accumulation session.
- PSUM channel pairing: c0+c1 share a [*,448] tile and one accumulation
  session (has_written bits make the split ranges exact); drains
  ping-pong between Act and DVE so neither sees the full burst rate.
- Outputs DMA per (slot, jb) half so the tail after the last matmul is
  one 112-row drain + a ~300KB DMA.
- Software pipelining: pass2 of slot s-1 is emitted after pass1 of slot
  s; samples are sorted by min(h,w) and dealt round-robin so slot s on
  every core holds same-sized windows (SPMD: one program, 8 cores).
"""

import sys
import os

for _p in ("/opt/trn_rl_repo",):
    if os.path.isdir(_p) and _p not in sys.path:
        sys.path.insert(0, _p)

import numpy as np

OUT_H = 224
OUT_W = 224
RESIZE_TO = np.float32(256.0)
B_FULL = 64
N_CORES = 8
B_LOC = B_FULL // N_CORES  # 8 slots per core
C = 3
H = 512
W = 512  # image width after stripping the metadata column (stored width 513)

LAST_EXEC_NS = None
LAST_RESULTS = None
_NC_CACHE = {}

USE_F16 = os.environ.get("CENTERCROP_F16", "1") == "1"


def _interp_matrices(h, w):
    """Full S [512, OUT_H], G [512, OUT_W] fp32 interpolation matrices,
    mirroring the reference fp32 math bit-for-bit."""
    f32 = np.float32
    h = f32(h)
    w = f32(w)
    min_dim = min(h, w)
    scale = RESIZE_TO / min_dim
    h_res = np.round(h * scale)
    w_res = np.round(w * scale)
    top = np.round((h_res - f32(OUT_H)) / f32(2.0))
    left = np.round((w_res - f32(OUT_W)) / f32(2.0))

    def axis_mat(n_out, offset, dim, dim_res, n_src):
        idx = np.arange(n_out, dtype=np.float32) + offset
        src = np.clip((idx + f32(0.5)) * dim / dim_res - f32(0.5),
                      f32(0.0), dim - f32(1.0))
        p0f = np.floor(src)
        frac = src - p0f
        imax = np.int32(dim) - 1
        p0 = np.clip(p0f.astype(np.int32), 0, imax)
        p1 = np.minimum(p0 + 1, imax)
        mat = np.zeros((n_src, n_out), np.float32)
        cols = np.arange(n_out)
        np.add.at(mat, (p0, cols), f32(1.0) - frac)
        np.add.at(mat, (p1, cols), frac)
        return mat

    S = axis_mat(OUT_H, top, h, h_res, H)
    G = axis_mat(OUT_W, left, w, w_res, W)
    return S, G


def _bands(mat, tile_rows, n_tiles):
    """Per-tile [lo, hi) columns with any nonzero; None if empty."""
    out = []
    for t in range(n_tiles):
        rows = mat[t * tile_rows:(t + 1) * tile_rows]
        nz = np.nonzero(rows.any(axis=0))[0]
        out.append(None if nz.size == 0 else (int(nz[0]), int(nz[-1]) + 1))
    return out


def _union_bands(band_lists):
    n = len(band_lists[0])
    out = []
    for t in range(n):
        los = [b[t][0] for b in band_lists if b[t] is not None]
        his = [b[t][1] for b in band_lists if b[t] is not None]
        out.append(None if not los else (min(los), max(his)))
    return out


def _band_offsets(bands):
    """Packed running offsets for non-empty bands; total width last."""
    offs = []
    off = 0
    for b in bands:
        if b is None:
            offs.append(None)
        else:
            offs.append(off)
            off += b[1] - b[0]
    return offs, off


def _prepare(x):
    """Host prep: per-sample windows/matrices, sorted slot assignment,
    per-core packed blobs, and the per-slot program parameters."""
    dtd_np = np.float16 if USE_F16 else np.float32
    h_all = x[:, 0, 0, -1].astype(np.float32)
    w_all = x[:, 1, 0, -1].astype(np.float32)

    samples = []
    for b in range(B_FULL):
        S, G = _interp_matrices(h_all[b], w_all[b])
        ynz = np.nonzero(S.any(axis=1))[0]
        xnz = np.nonzero(G.any(axis=1))[0]
        y0, y1 = int(ynz[0]), int(ynz[-1]) + 1
        x0, x1 = int(xnz[0]), int(xnz[-1]) + 1
        samples.append(dict(S=S[y0:y1], G=G[x0:x1], y0=y0, x0=x0,
                            wh=y1 - y0, ww=x1 - x0))

    order = np.argsort(np.minimum(h_all, w_all), kind="stable")
    # slot s, core c -> sample order[s*N_CORES + c]; slot index ascending
    # in window size.
    assign = [[int(order[s * N_CORES + c]) for c in range(N_CORES)]
              for s in range(B_LOC)]

    slot_params = []
    in_maps = [{} for _ in range(N_CORES)]
    for s in range(B_LOC):
        sids = assign[s]
        wh = max(samples[i]["wh"] for i in sids)
        ww = max(samples[i]["ww"] for i in sids)
        n_yt = (wh + 127) // 128
        n_xt = (ww + 127) // 128
        R = (wh + n_yt - 1) // n_yt   # balanced y-tile rows
        X = (ww + n_xt - 1) // n_xt   # balanced x-chunk cols
        # partition dim must be a multiple of 16: the HWDGE only shards a
        # transfer across the 16 SDMA engines at 16-row granularity; odd
        # partition counts degrade to single-engine processing.
        P = (max(R, X) + 15) // 16 * 16

        sb_list, gb_list, mats = [], [], []
        for i in sids:
            sp = samples[i]
            Sw = np.zeros((n_yt * R, OUT_H), np.float32)
            Sw[:sp["wh"]] = sp["S"]
            Gw = np.zeros((n_xt * X, OUT_W), np.float32)
            Gw[:sp["ww"]] = sp["G"]
            sb_list.append(_bands(Sw, R, n_yt))
            gb_list.append(_bands(Gw, X, n_xt))
            mats.append((Sw, Gw))
        sbands = tuple(_union_bands(sb_list))
        gbands = tuple(_union_bands(gb_list))
        s_offs, s_tot = _band_offsets(sbands)
        g_offs, g_tot = _band_offsets(gbands)
        img_w = C * n_yt * ww
        F = img_w + s_tot + g_tot
        slot_params.append((n_yt, n_xt, wh, ww, R, X, P,
                            sbands, gbands))

        for ci in range(N_CORES):
            sp = samples[sids[ci]]
            Sw, Gw = mats[ci]
            blob = np.zeros((P, F), dtd_np)
            win = x[sids[ci], :, sp["y0"]:sp["y0"] + sp["wh"],
                    sp["x0"]:sp["x0"] + sp["ww"]]
            for t in range(n_yt):
                rows = win[:, t * R:(t + 1) * R]  # [C, rt, ww_s]
                rt = rows.shape[1]
                for cch in range(C):
                    base = cch * n_yt * ww + t * ww
                    blob[:rt, base:base + sp["ww"]] = rows[cch]
            St = Sw.reshape(n_yt, R, OUT_H)
            Gt = Gw.reshape(n_xt, X, OUT_W)
            for t in range(n_yt):
                if sbands[t] is not None:
                    lo, hi = sbands[t]
                    o = img_w + s_offs[t]
                    blob[:R, o:o + hi - lo] = St[t, :, lo:hi]
            for xb in range(n_xt):
                if gbands[xb] is not None:
                    lo, hi = gbands[xb]
                    o = img_w + s_tot + g_offs[xb]
                    blob[:X, o:o + hi - lo] = Gt[xb, :, lo:hi]
            in_maps[ci][f"blob{s}"] = blob
    return tuple(slot_params), in_maps, assign


def _build_nc(slot_params):
    import concourse.bacc as bacc
    import concourse.mybir as mybir
    import concourse.tile as tile

    dt = mybir.dt.float32
    dtd = mybir.dt.float16 if USE_F16 else mybir.dt.float32
    nc = bacc.Bacc(
        "TRN2",
        target_bir_lowering=False,
        debug=False,
        enable_asserts=False,
        num_devices=N_CORES,
    )
    n_slots = len(slot_params)
    meta = []
    blob_in = []
    for s, (n_yt, n_xt, wh, ww, R, X, P, sbands, gbands) in \
            enumerate(slot_params):
        s_offs, s_tot = _band_offsets(sbands)
        g_offs, g_tot = _band_offsets(gbands)
        img_w = C * n_yt * ww
        F = img_w + s_tot + g_tot
        meta.append((s_offs, g_offs, img_w, img_w + s_tot, F))
        blob_in.append(nc.dram_tensor(f"blob{s}", [P, F], dtd,
                                      kind="ExternalInput"))
    out = nc.dram_tensor("out", [B_LOC, 2, 112, C, OUT_W], dtd,
                         kind="ExternalOutput")

    # head waits on slot_order[0]'s DMA; the tail drain is slot_order[-1]'s
    # pass2 + output DMA (keep both ends small)
    slot_order = list(range(1, n_slots)) + [0]

    with tile.TileContext(nc) as tc:
        with (
            tc.tile_pool(name="blob", bufs=n_slots) as blob_pool,
            tc.tile_pool(name="tmp", bufs=3) as tmp_pool,
            tc.tile_pool(name="outp", bufs=4) as out_pool,
            tc.tile_pool(name="ps1a", bufs=2, space="PSUM") as ps1a_pool,
            tc.tile_pool(name="ps1b", bufs=2, space="PSUM") as ps1b_pool,
            tc.tile_pool(name="ps2a", bufs=2, space="PSUM") as ps2a_pool,
            tc.tile_pool(name="ps2b", bufs=2, space="PSUM") as ps2b_pool,
        ):
            blob_tiles = [None] * n_slots
            tmp_tiles = [None] * n_slots

            # all input DMAs issued up front in compute order
            for s in slot_order:
                n_yt, n_xt, wh, ww, R, X, P, sbands, gbands = slot_params[s]
                F = meta[s][4]
                b_sb = blob_pool.tile([P, F], dtd)
                nc.sync.dma_start(b_sb[:], blob_in[s][:])
                blob_tiles[s] = b_sb

            def pass1(s):
                n_yt, n_xt, wh, ww, R, X, P, sbands, gbands = slot_params[s]
                s_offs, g_offs, img_w, g_base, F = meta[s]
                b_sb = blob_tiles[s]
                s_emit = [t for t in range(n_yt) if sbands[t] is not None]
                tmp_sb = tmp_pool.tile([128, n_xt, C, OUT_H], dtd)
                tmp_tiles[s] = tmp_sb
                for xb in range(n_xt):
                    xlo = xb * X
                    xn = min(X, ww - xlo)
                    ps1a = ps1a_pool.tile([128, 2 * OUT_H], dt, tag='ps1a')
                    ps1b = ps1b_pool.tile([128, OUT_H], dt, tag='ps1b')
                    # c0+c1 share one accumulation session in ps1a: PSUM
                    # has_written bits make the split ranges exact
                    for c in range(C):
                        base = c * OUT_H if c < 2 else 0
                        ps = ps1a if c < 2 else ps1b
                        for i_t, t in enumerate(s_emit):
                            lo, hi = sbands[t]
                            rt = min(R, wh - t * R)
                            ib = c * n_yt * ww + t * ww + xlo
                            o = img_w + s_offs[t]
                            first = i_t == 0 and (c == 0 or c == 2)
                            last = (i_t == len(s_emit) - 1
                                    and (c == 1 or c == 2))
                            nc.tensor.matmul(
                                ps[:xn, base + lo:base + hi],
                                b_sb[:rt, ib:ib + xn],
                                b_sb[:rt, o:o + hi - lo],
                                start=first,
                                stop=last,
                                skip_group_check=True,
                            )
                    # ping-pong the drains so each engine sees half the
                    # burst rate
                    if xb % 2 == 0:
                        nc.scalar.copy(tmp_sb[:xn, xb, 0:2, :],
                                       ps1a[:xn, :])
                        nc.vector.tensor_copy(tmp_sb[:xn, xb, 2, :],
                                              ps1b[:xn, :OUT_H])
                    else:
                        nc.vector.tensor_copy(tmp_sb[:xn, xb, 0:2, :],
                                              ps1a[:xn, :])
                        nc.scalar.copy(tmp_sb[:xn, xb, 2, :],
                                       ps1b[:xn, :OUT_H])

            def pass2(s):
                n_yt, n_xt, wh, ww, R, X, P, sbands, gbands = slot_params[s]
                s_offs, g_offs, img_w, g_base, F = meta[s]
                b_sb = blob_tiles[s]
                tmp_sb = tmp_tiles[s]
                g_emit = [t for t in range(n_xt) if gbands[t] is not None]
                for jb in range(2):
                    out_sb = out_pool.tile([112, C, OUT_W], dtd, tag="out")
                    ps2a = ps2a_pool.tile([112, 2 * OUT_W], dt, tag='ps2a')
                    ps2b = ps2b_pool.tile([112, OUT_W], dt, tag='ps2b')
                    for c in range(C):
                        base = c * OUT_W if c < 2 else 0
                        ps = ps2a if c < 2 else ps2b
                        for i_t, xb in enumerate(g_emit):
                            lo, hi = gbands[xb]
                            o = g_base + g_offs[xb]
                            xn = min(X, ww - xb * X)
                            first = i_t == 0 and (c == 0 or c == 2)
                            last = (i_t == len(g_emit) - 1
                                    and (c == 1 or c == 2))
                            nc.tensor.matmul(
                                ps[:, base + lo:base + hi],
                                tmp_sb[:xn, xb, c,
                                       jb * 112:(jb + 1) * 112],
                                b_sb[:xn, o:o + hi - lo],
                                start=first,
                                stop=last,
                                skip_group_check=True,
                            )
                    if jb == 0:
                        nc.vector.tensor_copy(out_sb[:, 0:2, :],
                                              ps2a[:, :])
                        nc.scalar.copy(out_sb[:, 2, :],
                                       ps2b[:, :OUT_W])
                    else:
                        nc.scalar.copy(out_sb[:, 0:2, :],
                                       ps2a[:, :])
                        nc.vector.tensor_copy(out_sb[:, 2, :],
                                              ps2b[:, :OUT_W])
                    nc.sync.dma_start(out[s, jb], out_sb[:])

            # software pipeline: pass2 of the previous slot is emitted after
            # pass1 of the current one, so PE never head-of-line blocks on
            # the ps1->tmp drains.
            prev = None
            for s in slot_order:
                pass1(s)
                if prev is not None:
                    pass2(prev)
                prev = s
            pass2(prev)
    nc.compile()
    return nc


def kernel(x, _trace=False):
    global LAST_EXEC_NS, LAST_RESULTS
    from concourse.bass_utils import run_bass_kernel_spmd

    x = np.ascontiguousarray(np.asarray(x), dtype=np.float32)
    assert x.shape == (B_FULL, C, H, W + 1), x.shape

    slot_params, in_maps, assign = _prepare(x)
    key = (slot_params, USE_F16)
    if key not in _NC_CACHE:
        _NC_CACHE[key] = _build_nc(slot_params)
    nc = _NC_CACHE[key]

    res = run_bass_kernel_spmd(nc, in_maps, list(range(N_CORES)), trace=_trace)
    LAST_EXEC_NS = res.exec_time_ns
    LAST_RESULTS = res

    out_full = np.empty((B_FULL, C, OUT_H, OUT_W), np.float32)
    for s in range(B_LOC):
        for c in range(N_CORES):
            # device layout [2, 112, C, OUT_W]; j = jb*112 + p
            arr = res.results[c]["out"][s].astype(np.float32)
            out_full[assign[s][c]] = arr.transpose(2, 0, 1, 3).reshape(
                C, OUT_H, OUT_W)
    return out_full


# revision 4
# speedup vs baseline: 1.8893x; 1.0030x over previous
"""Trainium2 Bass kernel for nn_CenterCrop: per-sample resize(short-side=256)
+ center-crop(224), bilinear, batch sharded over 8 NeuronCores.

Bilinear resize is separable: out = S^T @ img @ G with per-sample sparse
interpolation matrices S (vertical) and G (horizontal), built on the host
from the h/w metadata. The gather+lerp runs on the PE array as matmuls:
  pass1: tmp1_T[x, j] = sum_y img[y, x] * S[y, j]   (img tiles stationary)
  pass2: out[j, i]    = sum_x tmp1_T[x, j] * G[x, i] (tmp1 tiles stationary)

Perf notes (v4):
- fp16 data path end-to-end (bilinear abs-rel error ~8e-4, gate 2e-2).
- DMA-byte-bound (~360 GB/s/core over 16 SDMA engines). Two blobs per
  slot: blob1 = full 128-row y-tiles + all full-chunk S|G bands (zero
  padding), blob2 = the partial last y-tile + its S band + the tail-chunk
  G band, with partitions rounded up to 16 (the HWDGE only shards a
  transfer across SDMA engines at 16-row granularity; odd partition
  counts degrade to single-engine processing).
- y/x tiles stay at 128 (not balanced): the PE clock only ramps to 2.4
  GHz under sustained full-size (K=128, M=128) matmul load; 96-125-row
  tiles keep it at 1.2 GHz and cost more than the padding saves.
- pass2 splits OUT_H as 128+96 (not 112+112) so half its matmuls run
  with M=128.
- PSUM: four dedicated double-buffered pools (8 banks) so a drain never
  blocks the next accumulation session. c0+c1 share a [*,448] tile and
  one accumulation session (has_written bits make the split ranges
  exact); drains ping-pong between Act and DVE.
- Outputs DMA per (slot, j-half); all triggers on the sync queue
  (~565ns DMA_SEQ_TIME each, front-loaded for inputs).
- Software pipelining: pass2 of slot s-1 emitted after pass1 of slot s;
  samples sorted by min(h,w), dealt round-robin: slot s has same-sized
  windows on every core (SPMD: one program).
"""

import sys
import os

for _p in ("/opt/trn_rl_repo",):
    if os.path.isdir(_p) and _p not in sys.path:
        sys.path.insert(0, _p)

import numpy as np

OUT_H = 224
OUT_W = 224
RESIZE_TO = np.float32(256.0)
B_FULL = 64
N_CORES = 8
B_LOC = B_FULL // N_CORES  # 8 slots per core
C = 3
H = 512
W = 512  # image width after stripping the metadata column (stored width 513)
J0 = 128  # pass2 first j-half (M=128 keeps the PE clock up)
J1 = OUT_H - J0

LAST_EXEC_NS = None
LAST_RESULTS = None
_NC_CACHE = {}

USE_F16 = os.environ.get("CENTERCROP_F16", "1") == "1"


def _interp_matrices(h, w):
    """Full S [512, OUT_H], G [512, OUT_W] fp32 interpolation matrices,
    mirroring the reference fp32 math bit-for-bit."""
    f32 = np.float32
    h = f32(h)
    w = f32(w)
    min_dim = min(h, w)
    scale = RESIZE_TO / min_dim
    h_res = np.round(h * scale)
    w_res = np.round(w * scale)
    top = np.round((h_res - f32(OUT_H)) / f32(2.0))
    left = np.round((w_res - f32(OUT_W)) / f32(2.0))

    def axis_mat(n_out, offset, dim, dim_res, n_src):
        idx = np.arange(n_out, dtype=np.float32) + offset
        src = np.clip((idx + f32(0.5)) * dim / dim_res - f32(0.5),
                      f32(0.0), dim - f32(1.0))
        p0f = np.floor(src)
        frac = src - p0f
        imax = np.int32(dim) - 1
        p0 = np.clip(p0f.astype(np.int32), 0, imax)
        p1 = np.minimum(p0 + 1, imax)
        mat = np.zeros((n_src, n_out), np.float32)
        cols = np.arange(n_out)
        np.add.at(mat, (p0, cols), f32(1.0) - frac)
        np.add.at(mat, (p1, cols), frac)
        return mat

    S = axis_mat(OUT_H, top, h, h_res, H)
    G = axis_mat(OUT_W, left, w, w_res, W)
    return S, G


def _bands(mat, n_tiles):
    """Per-128-row-tile [lo, hi) columns with any nonzero; None if empty."""
    out = []
    for t in range(n_tiles):
        rows = mat[t * 128:(t + 1) * 128]
        nz = np.nonzero(rows.any(axis=0))[0]
        out.append(None if nz.size == 0 else (int(nz[0]), int(nz[-1]) + 1))
    return out


def _union_bands(band_lists):
    n = len(band_lists[0])
    out = []
    for t in range(n):
        los = [b[t][0] for b in band_lists if b[t] is not None]
        his = [b[t][1] for b in band_lists if b[t] is not None]
        out.append(None if not los else (min(los), max(his)))
    return out


def _band_offsets(bands, base=0):
    """Packed running offsets for non-empty bands; total width last."""
    offs = []
    off = base
    for b in bands:
        if b is None:
            offs.append(None)
        else:
            offs.append(off)
            off += b[1] - b[0]
    return offs, off


def _slot_geom(wh, ww):
    """Tile geometry for one slot (all derived from union wh/ww)."""
    n_yt = (wh + 127) // 128
    n_xt = (ww + 127) // 128
    rt = wh - 128 * (n_yt - 1)   # rows in last y-tile (1..128)
    xt = ww - 128 * (n_xt - 1)   # cols in last x-chunk (1..128)
    full_yt = n_yt if rt == 128 else n_yt - 1
    full_xt = n_xt if xt == 128 else n_xt - 1
    P2 = (max(rt if rt < 128 else 0, xt if xt < 128 else 0) + 15) // 16 * 16
    return n_yt, n_xt, rt, xt, full_yt, full_xt, P2


def _prepare(x):
    """Host prep: per-sample windows/matrices, sorted slot assignment,
    per-core packed blobs, and the per-slot program parameters."""
    dtd_np = np.float16 if USE_F16 else np.float32
    h_all = x[:, 0, 0, -1].astype(np.float32)
    w_all = x[:, 1, 0, -1].astype(np.float32)

    samples = []
    for b in range(B_FULL):
        S, G = _interp_matrices(h_all[b], w_all[b])
        ynz = np.nonzero(S.any(axis=1))[0]
        xnz = np.nonzero(G.any(axis=1))[0]
        y0, y1 = int(ynz[0]), int(ynz[-1]) + 1
        x0, x1 = int(xnz[0]), int(xnz[-1]) + 1
        samples.append(dict(S=S[y0:y1], G=G[x0:x1], y0=y0, x0=x0,
                            wh=y1 - y0, ww=x1 - x0))

    order = np.argsort(np.minimum(h_all, w_all), kind="stable")
    assign = [[int(order[s * N_CORES + c]) for c in range(N_CORES)]
              for s in range(B_LOC)]

    slot_params = []
    in_maps = [{} for _ in range(N_CORES)]
    for s in range(B_LOC):
        sids = assign[s]
        wh = max(samples[i]["wh"] for i in sids)
        ww = max(samples[i]["ww"] for i in sids)
        n_yt, n_xt, rt, xt, full_yt, full_xt, P2 = _slot_geom(wh, ww)

        sb_list, gb_list, mats = [], [], []
        for i in sids:
            sp = samples[i]
            Sw = np.zeros((n_yt * 128, OUT_H), np.float32)
            Sw[:sp["wh"]] = sp["S"]
            Gw = np.zeros((n_xt * 128, OUT_W), np.float32)
            Gw[:sp["ww"]] = sp["G"]
            sb_list.append(_bands(Sw, n_yt))
            gb_list.append(_bands(Gw, n_xt))
            mats.append((Sw, Gw))
        sbands = tuple(_union_bands(sb_list))
        gbands = tuple(_union_bands(gb_list))
        slot_params.append((wh, ww, sbands, gbands))

        # blob1 layout: [128, C*full_yt*ww | S bands t<full_yt | G bands
        # xb<full_xt]; blob2: [P2, C*ww | last S band | last G band]
        img1_w = C * full_yt * ww
        s1_offs, off = _band_offsets(sbands[:full_yt], img1_w)
        g1_offs, F1 = _band_offsets(gbands[:full_xt], off)
        img2_w = C * ww if full_yt < n_yt else 0
        s2_off = img2_w
        s2_w = ((sbands[-1][1] - sbands[-1][0])
                if full_yt < n_yt and sbands[-1] is not None else 0)
        g2_off = img2_w + s2_w
        g2_w = ((gbands[-1][1] - gbands[-1][0])
                if full_xt < n_xt and gbands[-1] is not None else 0)
        F2 = g2_off + g2_w

        for ci in range(N_CORES):
            sp = samples[sids[ci]]
            Sw, Gw = mats[ci]
            blob1 = np.zeros((128, F1), dtd_np)
            blob2 = np.zeros((P2, max(F2, 1)), dtd_np)
            win = x[sids[ci], :, sp["y0"]:sp["y0"] + sp["wh"],
                    sp["x0"]:sp["x0"] + sp["ww"]]
            for t in range(n_yt):
                rows = win[:, t * 128:(t + 1) * 128]  # [C, tr, ww_s]
                tr = rows.shape[1]
                for cch in range(C):
                    if t < full_yt:
                        base = cch * full_yt * ww + t * ww
                        blob1[:tr, base:base + sp["ww"]] = rows[cch]
                    else:
                        base = cch * ww
                        blob2[:tr, base:base + sp["ww"]] = rows[cch]
            St = Sw.reshape(n_yt, 128, OUT_H)
            Gt = Gw.reshape(n_xt, 128, OUT_W)
            for t in range(n_yt):
                if sbands[t] is None:
                    continue
                lo, hi = sbands[t]
                if t < full_yt:
                    o = s1_offs[t]
                    blob1[:, o:o + hi - lo] = St[t, :, lo:hi]
                else:
                    blob2[:rt, s2_off:s2_off + hi - lo] = St[t, :rt, lo:hi]
            for xb in range(n_xt):
                if gbands[xb] is None:
                    continue
                lo, hi = gbands[xb]
                if xb < full_xt:
                    o = g1_offs[xb]
                    blob1[:, o:o + hi - lo] = Gt[xb, :, lo:hi]
                else:
                    blob2[:xt, g2_off:g2_off + hi - lo] = Gt[xb, :xt, lo:hi]
            in_maps[ci][f"b1_{s}"] = blob1
            in_maps[ci][f"b2_{s}"] = blob2
    return tuple(slot_params), in_maps, assign


def _build_nc(slot_params):
    import concourse.bacc as bacc
    import concourse.mybir as mybir
    import concourse.tile as tile

    dt = mybir.dt.float32
    dtd = mybir.dt.float16 if USE_F16 else mybir.dt.float32
    nc = bacc.Bacc(
        "TRN2",
        target_bir_lowering=False,
        debug=False,
        enable_asserts=False,
        num_devices=N_CORES,
    )
    n_slots = len(slot_params)
    meta = []
    b1_in, b2_in = [], []
    for s, (wh, ww, sbands, gbands) in enumerate(slot_params):
        n_yt, n_xt, rt, xt, full_yt, full_xt, P2 = _slot_geom(wh, ww)
        img1_w = C * full_yt * ww
        s1_offs, off = _band_offsets(sbands[:full_yt], img1_w)
        g1_offs, F1 = _band_offsets(gbands[:full_xt], off)
        img2_w = C * ww if full_yt < n_yt else 0
        s2_off = img2_w
        s2_w = ((sbands[-1][1] - sbands[-1][0])
                if full_yt < n_yt and sbands[-1] is not None else 0)
        g2_off = img2_w + s2_w
        g2_w = ((gbands[-1][1] - gbands[-1][0])
                if full_xt < n_xt and gbands[-1] is not None else 0)
        F2 = g2_off + g2_w
        meta.append((n_yt, n_xt, rt, xt, full_yt, full_xt, P2,
                     s1_offs, g1_offs, F1, s2_off, g2_off, F2))
        b1_in.append(nc.dram_tensor(f"b1_{s}", [128, F1], dtd,
                                    kind="ExternalInput"))
        b2_in.append(nc.dram_tensor(f"b2_{s}", [P2, max(F2, 1)], dtd,
                                    kind="ExternalInput"))
    out = nc.dram_tensor("out", [B_LOC, OUT_H, C, OUT_W], dtd,
                         kind="ExternalOutput")

    slot_order = list(range(1, n_slots)) + [0]

    with tile.TileContext(nc) as tc:
        with (
            tc.tile_pool(name="b1", bufs=n_slots) as b1_pool,
            tc.tile_pool(name="b2", bufs=n_slots) as b2_pool,
            tc.tile_pool(name="tmp", bufs=3) as tmp_pool,
            tc.tile_pool(name="outp", bufs=4) as out_pool,
            tc.tile_pool(name="ps1a", bufs=2, space="PSUM") as ps1a_pool,
            tc.tile_pool(name="ps1b", bufs=2, space="PSUM") as ps1b_pool,
            tc.tile_pool(name="ps2a", bufs=2, space="PSUM") as ps2a_pool,
            tc.tile_pool(name="ps2b", bufs=2, space="PSUM") as ps2b_pool,
        ):
            b1_tiles = [None] * n_slots
            b2_tiles = [None] * n_slots
            tmp_tiles = [None] * n_slots

            for s in slot_order:
                (n_yt, n_xt, rt, xt, full_yt, full_xt, P2,
                 s1_offs, g1_offs, F1, s2_off, g2_off, F2) = meta[s]
                t1 = b1_pool.tile([128, F1], dtd)
                nc.sync.dma_start(t1[:], b1_in[s][:])
                b1_tiles[s] = t1
                if F2 > 0:
                    t2 = b2_pool.tile([P2, F2], dtd)
                    nc.sync.dma_start(t2[:], b2_in[s][:])
                    b2_tiles[s] = t2

            def pass1(s):
                wh, ww, sbands, gbands = slot_params[s]
                (n_yt, n_xt, rt, xt, full_yt, full_xt, P2,
                 s1_offs, g1_offs, F1, s2_off, g2_off, F2) = meta[s]
                b1 = b1_tiles[s]
                b2 = b2_tiles[s]
                s_emit = [t for t in range(n_yt) if sbands[t] is not None]
                tmp_sb = tmp_pool.tile([128, n_xt, C, OUT_H], dtd)
                tmp_tiles[s] = tmp_sb
                for xb in range(n_xt):
                    xlo = xb * 128
                    xn = min(128, ww - xlo)
                    ps1a = ps1a_pool.tile([128, 2 * OUT_H], dt, tag='ps1a')
                    ps1b = ps1b_pool.tile([128, OUT_H], dt, tag='ps1b')
                    for c in range(C):
                        base = c * OUT_H if c < 2 else 0
                        ps = ps1a if c < 2 else ps1b
                        for i_t, t in enumerate(s_emit):
                            lo, hi = sbands[t]
                            first = i_t == 0 and (c == 0 or c == 2)
                            last = (i_t == len(s_emit) - 1
                                    and (c == 1 or c == 2))
                            if t < full_yt:
                                ib = c * full_yt * ww + t * ww + xlo
                                w_ap = b1[:, ib:ib + xn]
                                s_ap = b1[:, s1_offs[t]:s1_offs[t] + hi - lo]
                            else:
                                ib = c * ww + xlo
                                w_ap = b2[:rt, ib:ib + xn]
                                s_ap = b2[:rt, s2_off:s2_off + hi - lo]
                            nc.tensor.matmul(
                                ps[:xn, base + lo:base + hi],
                                w_ap,
                                s_ap,
                                start=first,
                                stop=last,
                                skip_group_check=True,
                            )
                    if xb % 2 == 0:
                        nc.scalar.copy(tmp_sb[:xn, xb, 0:2, :],
                                       ps1a[:xn, :])
                        nc.vector.tensor_copy(tmp_sb[:xn, xb, 2, :],
                                              ps1b[:xn, :OUT_H])
                    else:
                        nc.vector.tensor_copy(tmp_sb[:xn, xb, 0:2, :],
                                              ps1a[:xn, :])
                        nc.scalar.copy(tmp_sb[:xn, xb, 2, :],
                                       ps1b[:xn, :OUT_H])

            def pass2(s):
                wh, ww, sbands, gbands = slot_params[s]
                (n_yt, n_xt, rt, xt, full_yt, full_xt, P2,
                 s1_offs, g1_offs, F1, s2_off, g2_off, F2) = meta[s]
                b1 = b1_tiles[s]
                b2 = b2_tiles[s]
                tmp_sb = tmp_tiles[s]
                g_emit = [t for t in range(n_xt) if gbands[t] is not None]
                for jb, (jlo, jn) in enumerate(((0, J0), (J0, J1))):
                    out_sb = out_pool.tile([jn, C, OUT_W], dtd, tag="out")
                    ps2a = ps2a_pool.tile([jn, 2 * OUT_W], dt, tag='ps2a')
                    ps2b = ps2b_pool.tile([jn, OUT_W], dt, tag='ps2b')
                    for c in range(C):
                        base = c * OUT_W if c < 2 else 0
                        ps = ps2a if c < 2 else ps2b
                        for i_t, xb in enumerate(g_emit):
                            lo, hi = gbands[xb]
                            xn = min(128, ww - xb * 128)
                            first = i_t == 0 and (c == 0 or c == 2)
                            last = (i_t == len(g_emit) - 1
                                    and (c == 1 or c == 2))
                            if xb < full_xt:
                                g_ap = b1[:xn, g1_offs[xb]:
                                          g1_offs[xb] + hi - lo]
                            else:
                                g_ap = b2[:xn, g2_off:g2_off + hi - lo]
                            nc.tensor.matmul(
                                ps[:, base + lo:base + hi],
                                tmp_sb[:xn, xb, c, jlo:jlo + jn],
                                g_ap,
                                start=first,
                                stop=last,
                                skip_group_check=True,
                            )
                    if jb == 0:
                        nc.vector.tensor_copy(out_sb[:, 0:2, :],
                                              ps2a[:, :])
                        nc.scalar.copy(out_sb[:, 2, :],
                                       ps2b[:, :OUT_W])
                    else:
                        nc.scalar.copy(out_sb[:, 0:2, :],
                                       ps2a[:, :])
                        nc.vector.tensor_copy(out_sb[:, 2, :],
                                              ps2b[:, :OUT_W])
                    nc.sync.dma_start(out[s, jlo:jlo + jn], out_sb[:])

            prev = None
            for s in slot_order:
                pass1(s)
                if prev is not None:
                    pass2(prev)
                prev = s
            pass2(prev)
    nc.compile()
    return nc


def kernel(x, _trace=False):
    global LAST_EXEC_NS, LAST_RESULTS
    from concourse.bass_utils import run_bass_kernel_spmd

    x = np.ascontiguousarray(np.asarray(x), dtype=np.float32)
    assert x.shape == (B_FULL, C, H, W + 1), x.shape

    slot_params, in_maps, assign = _prepare(x)
    key = (slot_params, USE_F16)
    if key not in _NC_CACHE:
        _NC_CACHE[key] = _build_nc(slot_params)
    nc = _NC_CACHE[key]

    res = run_bass_kernel_spmd(nc, in_maps, list(range(N_CORES)), trace=_trace)
    LAST_EXEC_NS = res.exec_time_ns
    LAST_RESULTS = res

    out_full = np.empty((B_FULL, C, OUT_H, OUT_W), np.float32)
    for s in range(B_LOC):
        for c in range(N_CORES):
            # device layout [OUT_H, C, OUT_W]
            arr = res.results[c]["out"][s].astype(np.float32)
            out_full[assign[s][c]] = arr.transpose(1, 0, 2)
    return out_full


# revision 7
# speedup vs baseline: 1.9990x; 1.0580x over previous
"""Trainium2 Bass kernel for nn_CenterCrop: per-sample resize(short-side=256)
+ center-crop(224), bilinear, batch sharded over 8 NeuronCores.

Bilinear resize is separable: out = S^T @ img @ G with per-sample sparse
interpolation matrices S (vertical) and G (horizontal), built on the host
from the h/w metadata. The gather+lerp runs on the PE array as matmuls:
  pass1: tmp1_T[x, j] = sum_y img[y, x] * S[y, j]   (img tiles stationary)
  pass2: out[j, i]    = sum_x tmp1_T[x, j] * G[x, i] (tmp1 tiles stationary)

Perf notes (v4):
- fp16 data path end-to-end (bilinear abs-rel error ~8e-4, gate 2e-2).
- DMA-byte-bound (~360 GB/s/core over 16 SDMA engines). Two blobs per
  slot: blob1 = full 128-row y-tiles + all full-chunk S|G bands (zero
  padding), blob2 = the partial last y-tile + its S band + the tail-chunk
  G band, with partitions rounded up to 16 (the HWDGE only shards a
  transfer across SDMA engines at 16-row granularity; odd partition
  counts degrade to single-engine processing).
- y/x tiles stay at 128 (not balanced): the PE clock only ramps to 2.4
  GHz under sustained full-size (K=128, M=128) matmul load; 96-125-row
  tiles keep it at 1.2 GHz and cost more than the padding saves.
- pass2 splits OUT_H as 128+96 (not 112+112) so half its matmuls run
  with M=128.
- PSUM: four dedicated double-buffered pools (8 banks) so a drain never
  blocks the next accumulation session. c0+c1 share a [*,448] tile and
  one accumulation session (has_written bits make the split ranges
  exact); drains ping-pong between Act and DVE.
- Outputs DMA per (slot, j-half); all triggers on the sync queue
  (~565ns DMA_SEQ_TIME each, front-loaded for inputs).
- Software pipelining: pass2 of slot s-1 emitted after pass1 of slot s;
  samples sorted by min(h,w), dealt round-robin: slot s has same-sized
  windows on every core (SPMD: one program).
"""

import sys
import os

for _p in ("/opt/trn_rl_repo",):
    if os.path.isdir(_p) and _p not in sys.path:
        sys.path.insert(0, _p)

import numpy as np

OUT_H = 224
OUT_W = 224
RESIZE_TO = np.float32(256.0)
B_FULL = 64
N_CORES = 8
B_LOC = B_FULL // N_CORES  # 8 slots per core
C = 3
H = 512
W = 512  # image width after stripping the metadata column (stored width 513)
J0 = 128  # pass2 first j-half (M=128 keeps the PE clock up)
J1 = OUT_H - J0

LAST_EXEC_NS = None
LAST_RESULTS = None
_NC_CACHE = {}

USE_F16 = os.environ.get("CENTERCROP_F16", "1") == "1"


def _interp_matrices(h, w):
    """Full S [512, OUT_H], G [512, OUT_W] fp32 interpolation matrices,
    mirroring the reference fp32 math bit-for-bit."""
    f32 = np.float32
    h = f32(h)
    w = f32(w)
    min_dim = min(h, w)
    scale = RESIZE_TO / min_dim
    h_res = np.round(h * scale)
    w_res = np.round(w * scale)
    top = np.round((h_res - f32(OUT_H)) / f32(2.0))
    left = np.round((w_res - f32(OUT_W)) / f32(2.0))

    def axis_mat(n_out, offset, dim, dim_res, n_src):
        idx = np.arange(n_out, dtype=np.float32) + offset
        src = np.clip((idx + f32(0.5)) * dim / dim_res - f32(0.5),
                      f32(0.0), dim - f32(1.0))
        p0f = np.floor(src)
        frac = src - p0f
        imax = np.int32(dim) - 1
        p0 = np.clip(p0f.astype(np.int32), 0, imax)
        p1 = np.minimum(p0 + 1, imax)
        mat = np.zeros((n_src, n_out), np.float32)
        cols = np.arange(n_out)
        np.add.at(mat, (p0, cols), f32(1.0) - frac)
        np.add.at(mat, (p1, cols), frac)
        return mat

    S = axis_mat(OUT_H, top, h, h_res, H)
    G = axis_mat(OUT_W, left, w, w_res, W)
    return S, G


def _bands(mat, n_tiles):
    """Per-128-row-tile [lo, hi) columns with any nonzero; None if empty."""
    out = []
    for t in range(n_tiles):
        rows = mat[t * 128:(t + 1) * 128]
        nz = np.nonzero(rows.any(axis=0))[0]
        out.append(None if nz.size == 0 else (int(nz[0]), int(nz[-1]) + 1))
    return out


def _union_bands(band_lists):
    n = len(band_lists[0])
    out = []
    for t in range(n):
        los = [b[t][0] for b in band_lists if b[t] is not None]
        his = [b[t][1] for b in band_lists if b[t] is not None]
        out.append(None if not los else (min(los), max(his)))
    return out


def _band_offsets(bands, base=0):
    """Packed running offsets for non-empty bands; total width last."""
    offs = []
    off = base
    for b in bands:
        if b is None:
            offs.append(None)
        else:
            offs.append(off)
            off += b[1] - b[0]
    return offs, off


def _slot_geom(wh, ww):
    """Tile geometry for one slot (all derived from union wh/ww)."""
    n_yt = (wh + 127) // 128
    n_xt = (ww + 127) // 128
    rt = wh - 128 * (n_yt - 1)   # rows in last y-tile (1..128)
    xt = ww - 128 * (n_xt - 1)   # cols in last x-chunk (1..128)
    full_yt = n_yt if rt == 128 else n_yt - 1
    full_xt = n_xt if xt == 128 else n_xt - 1
    P2 = (max(rt if rt < 128 else 0, xt if xt < 128 else 0) + 15) // 16 * 16
    return n_yt, n_xt, rt, xt, full_yt, full_xt, P2


def _prepare(x):
    """Host prep: per-sample windows/matrices, sorted slot assignment,
    per-core packed blobs, and the per-slot program parameters."""
    dtd_np = np.float16 if USE_F16 else np.float32
    h_all = x[:, 0, 0, -1].astype(np.float32)
    w_all = x[:, 1, 0, -1].astype(np.float32)

    samples = []
    for b in range(B_FULL):
        S, G = _interp_matrices(h_all[b], w_all[b])
        ynz = np.nonzero(S.any(axis=1))[0]
        xnz = np.nonzero(G.any(axis=1))[0]
        y0, y1 = int(ynz[0]), int(ynz[-1]) + 1
        x0, x1 = int(xnz[0]), int(xnz[-1]) + 1
        samples.append(dict(S=S[y0:y1], G=G[x0:x1], y0=y0, x0=x0,
                            wh=y1 - y0, ww=x1 - x0))

    order = np.argsort(np.minimum(h_all, w_all), kind="stable")
    assign = [[int(order[s * N_CORES + c]) for c in range(N_CORES)]
              for s in range(B_LOC)]

    slot_params = []
    in_maps = [{} for _ in range(N_CORES)]
    for s in range(B_LOC):
        sids = assign[s]
        wh = max(samples[i]["wh"] for i in sids)
        ww = max(samples[i]["ww"] for i in sids)
        n_yt, n_xt, rt, xt, full_yt, full_xt, P2 = _slot_geom(wh, ww)

        sb_list, gb_list, mats = [], [], []
        for i in sids:
            sp = samples[i]
            Sw = np.zeros((n_yt * 128, OUT_H), np.float32)
            Sw[:sp["wh"]] = sp["S"]
            Gw = np.zeros((n_xt * 128, OUT_W), np.float32)
            Gw[:sp["ww"]] = sp["G"]
            sb_list.append(_bands(Sw, n_yt))
            gb_list.append(_bands(Gw, n_xt))
            mats.append((Sw, Gw))
        sbands = tuple(_union_bands(sb_list))
        gbands = tuple(_union_bands(gb_list))
        slot_params.append((wh, ww, sbands, gbands))

        # blob1 layout: [128, C*full_yt*ww | S bands t<full_yt | G bands
        # xb<full_xt]; blob2: [P2, C*ww | last S band | last G band]
        img1_w = C * full_yt * ww
        s1_offs, off = _band_offsets(sbands[:full_yt], img1_w)
        g1_offs, F1 = _band_offsets(gbands[:full_xt], off)
        img2_w = C * ww if full_yt < n_yt else 0
        s2_off = img2_w
        s2_w = ((sbands[-1][1] - sbands[-1][0])
                if full_yt < n_yt and sbands[-1] is not None else 0)
        g2_off = img2_w + s2_w
        g2_w = ((gbands[-1][1] - gbands[-1][0])
                if full_xt < n_xt and gbands[-1] is not None else 0)
        F2 = g2_off + g2_w

        for ci in range(N_CORES):
            sp = samples[sids[ci]]
            Sw, Gw = mats[ci]
            blob1 = np.zeros((128, F1), dtd_np)
            blob2 = np.zeros((P2, max(F2, 1)), dtd_np)
            win = x[sids[ci], :, sp["y0"]:sp["y0"] + sp["wh"],
                    sp["x0"]:sp["x0"] + sp["ww"]]
            for t in range(n_yt):
                rows = win[:, t * 128:(t + 1) * 128]  # [C, tr, ww_s]
                tr = rows.shape[1]
                for cch in range(C):
                    if t < full_yt:
                        base = cch * full_yt * ww + t * ww
                        blob1[:tr, base:base + sp["ww"]] = rows[cch]
                    else:
                        base = cch * ww
                        blob2[:tr, base:base + sp["ww"]] = rows[cch]
            St = Sw.reshape(n_yt, 128, OUT_H)
            Gt = Gw.reshape(n_xt, 128, OUT_W)
            for t in range(n_yt):
                if sbands[t] is None:
                    continue
                lo, hi = sbands[t]
                if t < full_yt:
                    o = s1_offs[t]
                    blob1[:, o:o + hi - lo] = St[t, :, lo:hi]
                else:
                    blob2[:rt, s2_off:s2_off + hi - lo] = St[t, :rt, lo:hi]
            for xb in range(n_xt):
                if gbands[xb] is None:
                    continue
                lo, hi = gbands[xb]
                if xb < full_xt:
                    o = g1_offs[xb]
                    blob1[:, o:o + hi - lo] = Gt[xb, :, lo:hi]
                else:
                    blob2[:xt, g2_off:g2_off + hi - lo] = Gt[xb, :xt, lo:hi]
            in_maps[ci][f"b1_{s}"] = blob1
            in_maps[ci][f"b2_{s}"] = blob2
    return tuple(slot_params), in_maps, assign


def _build_nc(slot_params):
    import concourse.bacc as bacc
    import concourse.mybir as mybir
    import concourse.tile as tile

    dt = mybir.dt.float32
    dtd = mybir.dt.float16 if USE_F16 else mybir.dt.float32
    nc = bacc.Bacc(
        "TRN2",
        target_bir_lowering=False,
        debug=False,
        enable_asserts=False,
        num_devices=N_CORES,
    )
    n_slots = len(slot_params)
    meta = []
    b1_in, b2_in = [], []
    for s, (wh, ww, sbands, gbands) in enumerate(slot_params):
        n_yt, n_xt, rt, xt, full_yt, full_xt, P2 = _slot_geom(wh, ww)
        img1_w = C * full_yt * ww
        s1_offs, off = _band_offsets(sbands[:full_yt], img1_w)
        g1_offs, F1 = _band_offsets(gbands[:full_xt], off)
        img2_w = C * ww if full_yt < n_yt else 0
        s2_off = img2_w
        s2_w = ((sbands[-1][1] - sbands[-1][0])
                if full_yt < n_yt and sbands[-1] is not None else 0)
        g2_off = img2_w + s2_w
        g2_w = ((gbands[-1][1] - gbands[-1][0])
                if full_xt < n_xt and gbands[-1] is not None else 0)
        F2 = g2_off + g2_w
        meta.append((n_yt, n_xt, rt, xt, full_yt, full_xt, P2,
                     s1_offs, g1_offs, F1, s2_off, g2_off, F2))
        b1_in.append(nc.dram_tensor(f"b1_{s}", [128, F1], dtd,
                                    kind="ExternalInput"))
        b2_in.append(nc.dram_tensor(f"b2_{s}", [P2, max(F2, 1)], dtd,
                                    kind="ExternalInput"))
    out = nc.dram_tensor("out", [B_LOC, 112, 2, C, OUT_W], dtd,
                         kind="ExternalOutput")

    slot_order = list(range(1, n_slots)) + [0]

    with tile.TileContext(nc) as tc:
        with (
            tc.tile_pool(name="b1", bufs=n_slots) as b1_pool,
            tc.tile_pool(name="b2", bufs=n_slots) as b2_pool,
            tc.tile_pool(name="tmp", bufs=3) as tmp_pool,
            tc.tile_pool(name="outp", bufs=3) as out_pool,
            tc.tile_pool(name="ps1a", bufs=3, space="PSUM") as ps1a_pool,
            tc.tile_pool(name="ps1b", bufs=2, space="PSUM") as ps1b_pool,
            tc.tile_pool(name="ps2", bufs=3, space="PSUM") as ps2_pool,
        ):
            b1_tiles = [None] * n_slots
            b2_tiles = [None] * n_slots
            tmp_tiles = [None] * n_slots

            for s in slot_order:
                (n_yt, n_xt, rt, xt, full_yt, full_xt, P2,
                 s1_offs, g1_offs, F1, s2_off, g2_off, F2) = meta[s]
                t1 = b1_pool.tile([128, F1], dtd)
                nc.sync.dma_start(t1[:], b1_in[s][:])
                b1_tiles[s] = t1
                if F2 > 0:
                    t2 = b2_pool.tile([P2, F2], dtd)
                    nc.sync.dma_start(t2[:], b2_in[s][:])
                    b2_tiles[s] = t2

            def pass1(s):
                wh, ww, sbands, gbands = slot_params[s]
                (n_yt, n_xt, rt, xt, full_yt, full_xt, P2,
                 s1_offs, g1_offs, F1, s2_off, g2_off, F2) = meta[s]
                b1 = b1_tiles[s]
                b2 = b2_tiles[s]
                s_emit = [t for t in range(n_yt) if sbands[t] is not None]
                tmp_sb = tmp_pool.tile([128, n_xt, C, OUT_H], dtd)
                tmp_tiles[s] = tmp_sb
                for xb in range(n_xt):
                    xlo = xb * 128
                    xn = min(128, ww - xlo)
                    ps1a = ps1a_pool.tile([128, 2 * OUT_H], dt, tag='ps1a')
                    ps1b = ps1b_pool.tile([128, OUT_H], dt, tag='ps1b')
                    for c in range(C):
                        base = c * OUT_H if c < 2 else 0
                        ps = ps1a if c < 2 else ps1b
                        for i_t, t in enumerate(s_emit):
                            lo, hi = sbands[t]
                            first = i_t == 0 and (c == 0 or c == 2)
                            last = (i_t == len(s_emit) - 1
                                    and (c == 1 or c == 2))
                            if t < full_yt:
                                ib = c * full_yt * ww + t * ww + xlo
                                w_ap = b1[:, ib:ib + xn]
                                s_ap = b1[:, s1_offs[t]:s1_offs[t] + hi - lo]
                            else:
                                ib = c * ww + xlo
                                w_ap = b2[:rt, ib:ib + xn]
                                s_ap = b2[:rt, s2_off:s2_off + hi - lo]
                            nc.tensor.matmul(
                                ps[:xn, base + lo:base + hi],
                                w_ap,
                                s_ap,
                                start=first,
                                stop=last,
                                skip_group_check=True,
                            )
                    if xb % 2 == 0:
                        nc.scalar.copy(tmp_sb[:xn, xb, 0:2, :],
                                       ps1a[:xn, :])
                        nc.vector.tensor_copy(tmp_sb[:xn, xb, 2, :],
                                              ps1b[:xn, :OUT_H])
                    else:
                        nc.vector.tensor_copy(tmp_sb[:xn, xb, 0:2, :],
                                              ps1a[:xn, :])
                        nc.scalar.copy(tmp_sb[:xn, xb, 2, :],
                                       ps1b[:xn, :OUT_H])

            def pass2(s):
                wh, ww, sbands, gbands = slot_params[s]
                (n_yt, n_xt, rt, xt, full_yt, full_xt, P2,
                 s1_offs, g1_offs, F1, s2_off, g2_off, F2) = meta[s]
                b1 = b1_tiles[s]
                b2 = b2_tiles[s]
                tmp_sb = tmp_tiles[s]
                g_emit = [t for t in range(n_xt) if gbands[t] is not None]
                out_sb = out_pool.tile([112, 2, C, OUT_W], dtd, tag="out")
                for jb in range(2):
                    ps2a = ps2_pool.tile([112, 2 * OUT_W], dt, tag='ps2')
                    ps2b = ps2_pool.tile([112, OUT_W], dt, tag='ps2')
                    for c in range(C):
                        base = c * OUT_W if c < 2 else 0
                        ps = ps2a if c < 2 else ps2b
                        for i_t, xb in enumerate(g_emit):
                            lo, hi = gbands[xb]
                            xn = min(128, ww - xb * 128)
                            first = i_t == 0 and (c == 0 or c == 2)
                            last = (i_t == len(g_emit) - 1
                                    and (c == 1 or c == 2))
                            if xb < full_xt:
                                g_ap = b1[:xn, g1_offs[xb]:
                                          g1_offs[xb] + hi - lo]
                            else:
                                g_ap = b2[:xn, g2_off:g2_off + hi - lo]
                            nc.tensor.matmul(
                                ps[:, base + lo:base + hi],
                                tmp_sb[:xn, xb, c,
                                       jb * 112:(jb + 1) * 112],
                                g_ap,
                                start=first,
                                stop=last,
                                skip_group_check=True,
                            )
                    if jb == 0:
                        nc.vector.tensor_copy(out_sb[:, jb, 0:2, :],
                                              ps2a[:, :])
                        nc.scalar.copy(out_sb[:, jb, 2, :],
                                       ps2b[:, :OUT_W])
                    else:
                        nc.scalar.copy(out_sb[:, jb, 0:2, :],
                                       ps2a[:, :])
                        nc.vector.tensor_copy(out_sb[:, jb, 2, :],
                                              ps2b[:, :OUT_W])
                nc.sync.dma_start(out[s][:], out_sb[:])

            prev = None
            for s in slot_order:
                pass1(s)
                if prev is not None:
                    pass2(prev)
                prev = s
            pass2(prev)
    nc.compile()
    return nc


def kernel(x, _trace=False):
    global LAST_EXEC_NS, LAST_RESULTS
    from concourse.bass_utils import run_bass_kernel_spmd

    x = np.ascontiguousarray(np.asarray(x), dtype=np.float32)
    assert x.shape == (B_FULL, C, H, W + 1), x.shape

    slot_params, in_maps, assign = _prepare(x)
    key = (slot_params, USE_F16)
    if key not in _NC_CACHE:
        _NC_CACHE[key] = _build_nc(slot_params)
    nc = _NC_CACHE[key]

    res = run_bass_kernel_spmd(nc, in_maps, list(range(N_CORES)), trace=_trace)
    LAST_EXEC_NS = res.exec_time_ns
    LAST_RESULTS = res

    out_full = np.empty((B_FULL, C, OUT_H, OUT_W), np.float32)
    for s in range(B_LOC):
        for c in range(N_CORES):
            # device layout [112, 2, C, OUT_W]; j = jb*112 + p
            arr = res.results[c]["out"][s].astype(np.float32)
            out_full[assign[s][c]] = arr.transpose(2, 1, 0, 3).reshape(
                C, OUT_H, OUT_W)
    return out_full


# revision 12
# speedup vs baseline: 2.1178x; 1.0594x over previous
"""Trainium2 Bass kernel for nn_CenterCrop: per-sample resize(short-side=256)
+ center-crop(224), bilinear, batch sharded over 8 NeuronCores.

Bilinear resize is separable: out = S^T @ img @ G with per-sample sparse
interpolation matrices S (vertical) and G (horizontal), built on the host
from the h/w metadata. The gather+lerp runs on the PE array as matmuls:
  pass1: tmp1_T[x, j] = sum_y img[y, x] * S[y, j]   (img tiles stationary)
  pass2: out[j, i]    = sum_x tmp1_T[x, j] * G[x, i] (tmp1 tiles stationary)

Perf structure (baseline fp32 115.5us -> ~44us measured):
- fp16 data path end-to-end: PE streams at 1 cyc/row (4x the fp32 rate),
  DMA bytes halve, output written fp16 and upcast on host. Bilinear error
  stays ~8e-4 max-rel (gate is 2e-2).
- Only the per-sample source window that the output reads is DMA'd, in a
  partition-major layout so every DMA is one linear run per partition.
- S/G are banded (2 nonzeros per column); only the per-128-row-tile
  nonzero band columns ship, packed into two blobs (first two compute
  slots' bands alone so the first matmul isn't gated on the rest).
- All input DMAs are issued up front (img bufs = n_slots, no reuse WAR).
- PSUM channel pairing: c0+c1 share a [*,448] PSUM tile and a single
  accumulation session (has_written bits make split ranges exact), so
  each x-chunk drains in 2 wide casts instead of 3 narrow ones; the two
  drain engines (Act, DVE) ping-pong per x-chunk/jb so neither sees the
  full burst rate (GpSimd cannot touch PSUM, DMA cannot read it).
- Software pipelining: pass2 of slot s-1 is emitted after pass1 of slot
  s, so the PE queue never head-of-line blocks on the ps1->tmp drains.
- SPMD requires one program for all 8 cores, so samples are sorted by
  min(h,w) and dealt round-robin: slot s on every core holds same-sized
  windows; the program is specialized per-slot to the union shape/bands.
  Outputs are unpermuted/transposed back on the host.
"""

import sys
import os

for _p in ("/opt/trn_rl_repo",):
    if os.path.isdir(_p) and _p not in sys.path:
        sys.path.insert(0, _p)

import numpy as np

OUT_H = 224
OUT_W = 224
RESIZE_TO = np.float32(256.0)
B_FULL = 64
N_CORES = 8
B_LOC = B_FULL // N_CORES  # 8 slots per core
C = 3
H = 512
W = 512  # image width after stripping the metadata column (stored width 513)

LAST_EXEC_NS = None
LAST_RESULTS = None
_NC_CACHE = {}

# fp16 single-pass data path (default). Disable via CENTERCROP_F16=0 for an
# fp32 debugging fallback.
USE_F16 = os.environ.get("CENTERCROP_F16", "1") == "1"


def _interp_matrices(h, w):
    """Full S [512, OUT_H], G [512, OUT_W] fp32 interpolation matrices,
    mirroring the reference fp32 math bit-for-bit."""
    f32 = np.float32
    h = f32(h)
    w = f32(w)
    min_dim = min(h, w)
    scale = RESIZE_TO / min_dim
    h_res = np.round(h * scale)
    w_res = np.round(w * scale)
    top = np.round((h_res - f32(OUT_H)) / f32(2.0))
    left = np.round((w_res - f32(OUT_W)) / f32(2.0))

    def axis_mat(n_out, offset, dim, dim_res, n_src):
        idx = np.arange(n_out, dtype=np.float32) + offset
        src = np.clip((idx + f32(0.5)) * dim / dim_res - f32(0.5),
                      f32(0.0), dim - f32(1.0))
        p0f = np.floor(src)
        frac = src - p0f
        imax = np.int32(dim) - 1
        p0 = np.clip(p0f.astype(np.int32), 0, imax)
        p1 = np.minimum(p0 + 1, imax)
        mat = np.zeros((n_src, n_out), np.float32)
        cols = np.arange(n_out)
        np.add.at(mat, (p0, cols), f32(1.0) - frac)
        np.add.at(mat, (p1, cols), frac)
        return mat

    S = axis_mat(OUT_H, top, h, h_res, H)
    G = axis_mat(OUT_W, left, w, w_res, W)
    return S, G


def _bands(mat_w, n_tiles, tile_rows=128):
    """Per-tile [lo, hi) columns with any nonzero; None if empty."""
    out = []
    for t in range(n_tiles):
        rows = mat_w[t * tile_rows:(t + 1) * tile_rows]
        nz = np.nonzero(rows.any(axis=0))[0]
        out.append(None if nz.size == 0 else (int(nz[0]), int(nz[-1]) + 1))
    return out


def _union_bands(band_lists):
    n = len(band_lists[0])
    out = []
    for t in range(n):
        los = [b[t][0] for b in band_lists if b[t] is not None]
        his = [b[t][1] for b in band_lists if b[t] is not None]
        out.append(None if not los else (min(los), max(his)))
    return out


def _band_offsets(bands):
    """Packed running offsets for non-empty bands; total width last."""
    offs = []
    off = 0
    for b in bands:
        if b is None:
            offs.append(None)
        else:
            offs.append(off)
            off += b[1] - b[0]
    return offs, off


def _prepare(x):
    """Host prep: per-sample windows/matrices, sorted slot assignment,
    per-core packed inputs, and the per-slot program parameters."""
    dtd_np = np.float16 if USE_F16 else np.float32
    h_all = x[:, 0, 0, -1].astype(np.float32)
    w_all = x[:, 1, 0, -1].astype(np.float32)

    samples = []
    for b in range(B_FULL):
        S, G = _interp_matrices(h_all[b], w_all[b])
        ynz = np.nonzero(S.any(axis=1))[0]
        xnz = np.nonzero(G.any(axis=1))[0]
        y0, y1 = int(ynz[0]), int(ynz[-1]) + 1
        x0, x1 = int(xnz[0]), int(xnz[-1]) + 1
        samples.append(dict(S=S[y0:y1], G=G[x0:x1], y0=y0, x0=x0,
                            wh=y1 - y0, ww=x1 - x0))

    order = np.argsort(np.minimum(h_all, w_all), kind="stable")
    # slot s, core c -> sample order[s*N_CORES + c]
    assign = [[int(order[s * N_CORES + c]) for c in range(N_CORES)]
              for s in range(B_LOC)]

    slot_params = []
    slot_data = []  # per slot: list over cores of (sid, Sw_pad, Gw_pad)
    for s in range(B_LOC):
        sids = assign[s]
        wh = max(samples[i]["wh"] for i in sids)
        ww = max(samples[i]["ww"] for i in sids)
        n_yt = (wh + 127) // 128
        n_xt = (ww + 127) // 128
        # balanced x-chunks: the stationary-load (LDWEIGHTS) chain paces
        # the PE at max(xn, band) cols per matmul, so a 128,128,...,thin
        # split wastes LW columns vs even chunks of X = ceil(ww/n_xt).
        X = (ww + n_xt - 1) // n_xt
        sb_list, gb_list, data = [], [], []
        for i in sids:
            sp = samples[i]
            Sw = np.zeros((n_yt * 128, OUT_H), np.float32)
            Sw[:sp["wh"]] = sp["S"]
            Gw = np.zeros((n_xt * X, OUT_W), np.float32)
            Gw[:sp["ww"]] = sp["G"]
            sb_list.append(_bands(Sw, n_yt))
            gb_list.append(_bands(Gw, n_xt, X))
            data.append((i, Sw, Gw))
        sbands = _union_bands(sb_list)
        gbands = _union_bands(gb_list)
        slot_params.append((n_yt, n_xt, ww, X,
                            tuple(sbands), tuple(gbands)))
        slot_data.append(data)

    # pack per-core input maps; all slots' S|G bands share one "sgall"
    in_maps = [{} for _ in range(N_CORES)]
    sg_parts = [[] for _ in range(N_CORES)]
    for s in range(B_LOC):
        n_yt, n_xt, ww, X, sbands, gbands = slot_params[s]
        s_offs, s_tot = _band_offsets(sbands)
        g_offs, g_tot = _band_offsets(gbands)
        for c in range(N_CORES):
            sid, Sw, Gw = slot_data[s][c]
            sp = samples[sid]
            xw = np.zeros((C, n_yt, 128, ww), dtd_np)
            win = x[sid, :, sp["y0"]:sp["y0"] + sp["wh"],
                    sp["x0"]:sp["x0"] + sp["ww"]]
            for t in range(n_yt):
                rows = win[:, t * 128:(t + 1) * 128]
                xw[:, t, :rows.shape[1], :sp["ww"]] = rows
            # partition-major layout so the DMA is one linear run/partition
            xw = np.ascontiguousarray(xw.transpose(2, 0, 1, 3))
            # packed S|G bands: [128, s_tot + g_tot]
            sg = np.zeros((128, s_tot + g_tot), dtd_np)
            St = Sw.reshape(n_yt, 128, OUT_H)
            Gt = Gw.reshape(n_xt, X, OUT_W)
            for t in range(n_yt):
                if sbands[t] is not None:
                    lo, hi = sbands[t]
                    sg[:, s_offs[t]:s_offs[t] + hi - lo] = St[t, :, lo:hi]
            for t in range(n_xt):
                if gbands[t] is not None:
                    lo, hi = gbands[t]
                    o = s_tot + g_offs[t]
                    sg[:X, o:o + hi - lo] = Gt[t, :, lo:hi]
            in_maps[c][f"xw{s}"] = xw
            sg_parts[c].append(sg)
    order = list(range(1, B_LOC)) + [0]
    for c in range(N_CORES):
        in_maps[c]["sg_a"] = sg_parts[c][order[0]]
        in_maps[c]["sg_b"] = sg_parts[c][order[1]]
        in_maps[c]["sg_rest"] = np.concatenate(
            [sg_parts[c][s] for s in order[2:]], axis=1)
    return tuple(slot_params), in_maps, assign


def _build_nc(slot_params):
    import concourse.bacc as bacc
    import concourse.mybir as mybir
    import concourse.tile as tile

    dt = mybir.dt.float32
    dtd = mybir.dt.float16 if USE_F16 else mybir.dt.float32
    nc = bacc.Bacc(
        "TRN2",
        target_bir_lowering=False,
        debug=False,
        enable_asserts=False,
        num_devices=N_CORES,
    )
    xw_in, sg_meta = [], []
    for s, (n_yt, n_xt, ww, X, sbands, gbands) in enumerate(slot_params):
        s_offs, s_tot = _band_offsets(sbands)
        g_offs, g_tot = _band_offsets(gbands)
        sg_meta.append((s_offs, g_offs, s_tot, g_tot))
        xw_in.append(nc.dram_tensor(f"xw{s}", [128, C, n_yt, ww], dtd,
                                    kind="ExternalInput"))
    out = nc.dram_tensor("out", [B_LOC, 112, 2, C, OUT_W], dtd,
                         kind="ExternalOutput")

    n_slots = len(slot_params)
    # head waits on slot_order[0]'s DMA (keep it small-ish); the tail drain
    # is slot_order[-1]'s pass2 (make it the smallest slot)
    slot_order = list(range(1, n_slots)) + [0]
    # S|G bands ship as three blobs: the first compute slot's piece alone
    # (gates the very first matmul), the second slot's piece, then the rest
    sg_w = [sg_meta[s][2] + sg_meta[s][3] for s in range(n_slots)]
    sg_base = {slot_order[0]: 0, slot_order[1]: 0}
    off = 0
    for s in slot_order[2:]:
        sg_base[s] = off
        off += sg_w[s]
    sga_in = nc.dram_tensor("sg_a", [128, sg_w[slot_order[0]]], dtd,
                            kind="ExternalInput")
    sgb_in = nc.dram_tensor("sg_b", [128, sg_w[slot_order[1]]], dtd,
                            kind="ExternalInput")
    sgr_in = nc.dram_tensor("sg_rest", [128, off], dtd,
                            kind="ExternalInput")
    with tile.TileContext(nc) as tc:
        with (
            tc.tile_pool(name="img", bufs=n_slots) as img_pool,
            tc.tile_pool(name="sg", bufs=3) as sg_pool,
            tc.tile_pool(name="tmp", bufs=3) as tmp_pool,
            tc.tile_pool(name="outp", bufs=3) as out_pool,
            tc.tile_pool(name="ps1a", bufs=3, space="PSUM") as ps1a_pool,
            tc.tile_pool(name="ps1b", bufs=2, space="PSUM") as ps1b_pool,
            tc.tile_pool(name="ps2", bufs=3, space="PSUM") as ps2_pool,
        ):
            tmp_tiles = [None] * n_slots

            # prefetch everything up front: first slot's S|G, its image,
            # then the remaining S|G blob and the other images
            img_tiles = [None] * n_slots

            def img_dma(s):
                n_yt, n_xt, ww, X, _, _ = slot_params[s]
                img_sb = img_pool.tile([128, C, n_yt, ww], dtd)
                nc.sync.dma_start(img_sb[:], xw_in[s][:])
                img_tiles[s] = img_sb

            sga_sb = sg_pool.tile([128, sg_w[slot_order[0]]], dtd,
                                  tag="sga")
            nc.sync.dma_start(sga_sb[:], sga_in[:])
            img_dma(slot_order[0])
            sgb_sb = sg_pool.tile([128, sg_w[slot_order[1]]], dtd,
                                  tag="sgb")
            nc.sync.dma_start(sgb_sb[:], sgb_in[:])
            img_dma(slot_order[1])
            sgr_sb = sg_pool.tile([128, off], dtd, tag="sgr")
            nc.sync.dma_start(sgr_sb[:], sgr_in[:])
            for s in slot_order[2:]:
                img_dma(s)
            sg_tiles = [sgr_sb] * n_slots
            sg_tiles[slot_order[0]] = sga_sb
            sg_tiles[slot_order[1]] = sgb_sb

            def pass1(s):
                n_yt, n_xt, ww, X, sbands, gbands = slot_params[s]
                s_offs, g_offs, s_tot, g_tot = sg_meta[s]
                sg_sb = sg_tiles[s]
                img_sb = img_tiles[s]
                s_emit = [t for t in range(n_yt) if sbands[t] is not None]
                tmp_sb = tmp_pool.tile([128, n_xt, C, OUT_H], dtd)
                tmp_tiles[s] = tmp_sb
                for xb in range(n_xt):
                    xlo = xb * X
                    xn = min(X, ww - xlo)
                    ps1a = ps1a_pool.tile([128, 2 * OUT_H], dt, tag='ps1a')
                    ps1b = ps1b_pool.tile([128, OUT_H], dt, tag='ps1b')
                    # c0+c1 share one accumulation session in ps1a: PSUM
                    # has_written bits make the split ranges exact
                    for c in range(C):
                        base = c * OUT_H if c < 2 else 0
                        ps = ps1a if c < 2 else ps1b
                        for i_t, t in enumerate(s_emit):
                            lo, hi = sbands[t]
                            o = sg_base[s] + s_offs[t]
                            first = i_t == 0 and (c == 0 or c == 2)
                            last = (i_t == len(s_emit) - 1
                                    and (c == 1 or c == 2))
                            nc.tensor.matmul(
                                ps[:xn, base + lo:base + hi],
                                img_sb[:, c, t, xlo:xlo + xn],
                                sg_sb[:, o:o + hi - lo],
                                start=first,
                                stop=last,
                                skip_group_check=True,
                            )
                    # ping-pong the drains so each engine sees half the
                    # burst rate (small slots outrun a single drain engine)
                    if xb % 2 == 0:
                        nc.scalar.copy(tmp_sb[:xn, xb, 0:2, :],
                                       ps1a[:xn, :])
                        nc.vector.tensor_copy(tmp_sb[:xn, xb, 2, :],
                                              ps1b[:xn, :OUT_H])
                    else:
                        nc.vector.tensor_copy(tmp_sb[:xn, xb, 0:2, :],
                                              ps1a[:xn, :])
                        nc.scalar.copy(tmp_sb[:xn, xb, 2, :],
                                       ps1b[:xn, :OUT_H])

            def pass2(s):
                n_yt, n_xt, ww, X, sbands, gbands = slot_params[s]
                s_offs, g_offs, s_tot, g_tot = sg_meta[s]
                sg_sb = sg_tiles[s]
                tmp_sb = tmp_tiles[s]
                g_emit = [t for t in range(n_xt) if gbands[t] is not None]
                out_sb = out_pool.tile([112, 2, C, OUT_W], dtd, tag="out")
                for jb in range(2):
                    ps2a = ps2_pool.tile([112, 2 * OUT_W], dt, tag='ps2')
                    ps2b = ps2_pool.tile([112, OUT_W], dt, tag='ps2')
                    for c in range(C):
                        base = c * OUT_W if c < 2 else 0
                        ps = ps2a if c < 2 else ps2b
                        for i_t, xb in enumerate(g_emit):
                            lo, hi = gbands[xb]
                            o = sg_base[s] + s_tot + g_offs[xb]
                            xn = min(X, ww - xb * X)
                            first = i_t == 0 and (c == 0 or c == 2)
                            last = (i_t == len(g_emit) - 1
                                    and (c == 1 or c == 2))
                            nc.tensor.matmul(
                                ps[:, base + lo:base + hi],
                                tmp_sb[:xn, xb, c,
                                       jb * 112:(jb + 1) * 112],
                                sg_sb[:xn, o:o + hi - lo],
                                start=first,
                                stop=last,
                                skip_group_check=True,
                            )
                    if jb == 0:
                        nc.vector.tensor_copy(out_sb[:, jb, 0:2, :],
                                              ps2a[:, :])
                        nc.scalar.copy(out_sb[:, jb, 2, :],
                                       ps2b[:, :OUT_W])
                    else:
                        nc.scalar.copy(out_sb[:, jb, 0:2, :],
                                       ps2a[:, :])
                        nc.vector.tensor_copy(out_sb[:, jb, 2, :],
                                              ps2b[:, :OUT_W])
                nc.sync.dma_start(out[s][:], out_sb[:])

            # software pipeline: pass2 of the previous slot is emitted after
            # pass1 of the current one, so PE never head-of-line blocks on
            # the ps1->tmp drains.
            prev = None
            for s in slot_order:
                pass1(s)
                if prev is not None:
                    pass2(prev)
                prev = s
            pass2(prev)
    nc.compile()
    return nc


def kernel(x, _trace=False):
    global LAST_EXEC_NS, LAST_RESULTS
    from concourse.bass_utils import run_bass_kernel_spmd

    x = np.ascontiguousarray(np.asarray(x), dtype=np.float32)
    assert x.shape == (B_FULL, C, H, W + 1), x.shape

    slot_params, in_maps, assign = _prepare(x)
    key = (slot_params, USE_F16)
    if key not in _NC_CACHE:
        _NC_CACHE[key] = _build_nc(slot_params)
    nc = _NC_CACHE[key]

    res = run_bass_kernel_spmd(nc, in_maps, list(range(N_CORES)), trace=_trace)
    LAST_EXEC_NS = res.exec_time_ns
    LAST_RESULTS = res

    out_full = np.empty((B_FULL, C, OUT_H, OUT_W), np.float32)
    for s in range(B_LOC):
        for c in range(N_CORES):
            # device layout [112, 2, C, OUT_W]; j = b*112 + p
            arr = res.results[c]["out"][s].astype(np.float32)
            out_full[assign[s][c]] = arr.transpose(2, 1, 0, 3).reshape(
                C, OUT_H, OUT_W)
    return out_full



# revision 13
# speedup vs baseline: 2.1248x; 1.0033x over previous
"""Trainium2 Bass kernel for nn_CenterCrop: per-sample resize(short-side=256)
+ center-crop(224), bilinear, batch sharded over 8 NeuronCores.

Bilinear resize is separable: out = S^T @ img @ G with per-sample sparse
interpolation matrices S (vertical) and G (horizontal), built on the host
from the h/w metadata. The gather+lerp runs on the PE array as matmuls:
  pass1: tmp1_T[x, j] = sum_y img[y, x] * S[y, j]   (img tiles stationary)
  pass2: out[j, i]    = sum_x tmp1_T[x, j] * G[x, i] (tmp1 tiles stationary)

Perf structure (baseline fp32 115.5us -> ~44us measured):
- fp16 data path end-to-end: PE streams at 1 cyc/row (4x the fp32 rate),
  DMA bytes halve, output written fp16 and upcast on host. Bilinear error
  stays ~8e-4 max-rel (gate is 2e-2).
- Only the per-sample source window that the output reads is DMA'd, in a
  partition-major layout so every DMA is one linear run per partition.
- S/G are banded (2 nonzeros per column); only the per-128-row-tile
  nonzero band columns ship, packed into two blobs (first two compute
  slots' bands alone so the first matmul isn't gated on the rest).
- All input DMAs are issued up front (img bufs = n_slots, no reuse WAR).
- PSUM channel pairing: c0+c1 share a [*,448] PSUM tile and a single
  accumulation session (has_written bits make split ranges exact), so
  each x-chunk drains in 2 wide casts instead of 3 narrow ones; the two
  drain engines (Act, DVE) ping-pong per x-chunk/jb so neither sees the
  full burst rate (GpSimd cannot touch PSUM, DMA cannot read it).
- Software pipelining: pass2 of slot s-1 is emitted after pass1 of slot
  s, so the PE queue never head-of-line blocks on the ps1->tmp drains.
- SPMD requires one program for all 8 cores, so samples are sorted by
  min(h,w) and dealt round-robin: slot s on every core holds same-sized
  windows; the program is specialized per-slot to the union shape/bands.
  Outputs are unpermuted/transposed back on the host.
"""

import sys
import os

for _p in ("/opt/trn_rl_repo",):
    if os.path.isdir(_p) and _p not in sys.path:
        sys.path.insert(0, _p)

import numpy as np

OUT_H = 224
OUT_W = 224
RESIZE_TO = np.float32(256.0)
B_FULL = 64
N_CORES = 8
B_LOC = B_FULL // N_CORES  # 8 slots per core
C = 3
H = 512
W = 512  # image width after stripping the metadata column (stored width 513)

LAST_EXEC_NS = None
LAST_RESULTS = None
_NC_CACHE = {}

# fp16 single-pass data path (default). Disable via CENTERCROP_F16=0 for an
# fp32 debugging fallback.
USE_F16 = os.environ.get("CENTERCROP_F16", "1") == "1"


def _interp_matrices(h, w):
    """Full S [512, OUT_H], G [512, OUT_W] fp32 interpolation matrices,
    mirroring the reference fp32 math bit-for-bit."""
    f32 = np.float32
    h = f32(h)
    w = f32(w)
    min_dim = min(h, w)
    scale = RESIZE_TO / min_dim
    h_res = np.round(h * scale)
    w_res = np.round(w * scale)
    top = np.round((h_res - f32(OUT_H)) / f32(2.0))
    left = np.round((w_res - f32(OUT_W)) / f32(2.0))

    def axis_mat(n_out, offset, dim, dim_res, n_src):
        idx = np.arange(n_out, dtype=np.float32) + offset
        src = np.clip((idx + f32(0.5)) * dim / dim_res - f32(0.5),
                      f32(0.0), dim - f32(1.0))
        p0f = np.floor(src)
        frac = src - p0f
        imax = np.int32(dim) - 1
        p0 = np.clip(p0f.astype(np.int32), 0, imax)
        p1 = np.minimum(p0 + 1, imax)
        mat = np.zeros((n_src, n_out), np.float32)
        cols = np.arange(n_out)
        np.add.at(mat, (p0, cols), f32(1.0) - frac)
        np.add.at(mat, (p1, cols), frac)
        return mat

    S = axis_mat(OUT_H, top, h, h_res, H)
    G = axis_mat(OUT_W, left, w, w_res, W)
    return S, G


def _bands(mat_w, n_tiles, tile_rows=128):
    """Per-tile [lo, hi) columns with any nonzero; None if empty."""
    out = []
    for t in range(n_tiles):
        rows = mat_w[t * tile_rows:(t + 1) * tile_rows]
        nz = np.nonzero(rows.any(axis=0))[0]
        out.append(None if nz.size == 0 else (int(nz[0]), int(nz[-1]) + 1))
    return out


def _union_bands(band_lists):
    n = len(band_lists[0])
    out = []
    for t in range(n):
        los = [b[t][0] for b in band_lists if b[t] is not None]
        his = [b[t][1] for b in band_lists if b[t] is not None]
        out.append(None if not los else (min(los), max(his)))
    return out


def _band_offsets(bands):
    """Packed running offsets for non-empty bands; total width last."""
    offs = []
    off = 0
    for b in bands:
        if b is None:
            offs.append(None)
        else:
            offs.append(off)
            off += b[1] - b[0]
    return offs, off


def _prepare(x):
    """Host prep: per-sample windows/matrices, sorted slot assignment,
    per-core packed inputs, and the per-slot program parameters."""
    dtd_np = np.float16 if USE_F16 else np.float32
    h_all = x[:, 0, 0, -1].astype(np.float32)
    w_all = x[:, 1, 0, -1].astype(np.float32)

    samples = []
    for b in range(B_FULL):
        S, G = _interp_matrices(h_all[b], w_all[b])
        ynz = np.nonzero(S.any(axis=1))[0]
        xnz = np.nonzero(G.any(axis=1))[0]
        y0, y1 = int(ynz[0]), int(ynz[-1]) + 1
        x0, x1 = int(xnz[0]), int(xnz[-1]) + 1
        samples.append(dict(S=S[y0:y1], G=G[x0:x1], y0=y0, x0=x0,
                            wh=y1 - y0, ww=x1 - x0))

    order = np.argsort(np.minimum(h_all, w_all), kind="stable")
    # slot s, core c -> sample order[s*N_CORES + c]
    assign = [[int(order[s * N_CORES + c]) for c in range(N_CORES)]
              for s in range(B_LOC)]

    slot_params = []
    slot_data = []  # per slot: list over cores of (sid, Sw_pad, Gw_pad)
    for s in range(B_LOC):
        sids = assign[s]
        wh = max(samples[i]["wh"] for i in sids)
        ww = max(samples[i]["ww"] for i in sids)
        n_yt = (wh + 127) // 128
        n_xt = (ww + 127) // 128
        # balanced x-chunks: the stationary-load (LDWEIGHTS) chain paces
        # the PE at max(xn, band) cols per matmul, so a 128,128,...,thin
        # split wastes LW columns vs even chunks of X = ceil(ww/n_xt).
        X = (ww + n_xt - 1) // n_xt
        sb_list, gb_list, data = [], [], []
        for i in sids:
            sp = samples[i]
            Sw = np.zeros((n_yt * 128, OUT_H), np.float32)
            Sw[:sp["wh"]] = sp["S"]
            Gw = np.zeros((n_xt * X, OUT_W), np.float32)
            Gw[:sp["ww"]] = sp["G"]
            sb_list.append(_bands(Sw, n_yt))
            gb_list.append(_bands(Gw, n_xt, X))
            data.append((i, Sw, Gw))
        sbands = _union_bands(sb_list)
        gbands = _union_bands(gb_list)
        slot_params.append((n_yt, n_xt, ww, X,
                            tuple(sbands), tuple(gbands)))
        slot_data.append(data)

    # pack per-core input maps; all slots' S|G bands share one "sgall"
    in_maps = [{} for _ in range(N_CORES)]
    sg_parts = [[] for _ in range(N_CORES)]
    for s in range(B_LOC):
        n_yt, n_xt, ww, X, sbands, gbands = slot_params[s]
        s_offs, s_tot = _band_offsets(sbands)
        g_offs, g_tot = _band_offsets(gbands)
        for c in range(N_CORES):
            sid, Sw, Gw = slot_data[s][c]
            sp = samples[sid]
            xw = np.zeros((C, n_yt, 128, ww), dtd_np)
            win = x[sid, :, sp["y0"]:sp["y0"] + sp["wh"],
                    sp["x0"]:sp["x0"] + sp["ww"]]
            for t in range(n_yt):
                rows = win[:, t * 128:(t + 1) * 128]
                xw[:, t, :rows.shape[1], :sp["ww"]] = rows
            # partition-major layout so the DMA is one linear run/partition
            xw = np.ascontiguousarray(xw.transpose(2, 0, 1, 3))
            # packed S|G bands: [128, s_tot + g_tot]
            sg = np.zeros((128, s_tot + g_tot), dtd_np)
            St = Sw.reshape(n_yt, 128, OUT_H)
            Gt = Gw.reshape(n_xt, X, OUT_W)
            for t in range(n_yt):
                if sbands[t] is not None:
                    lo, hi = sbands[t]
                    sg[:, s_offs[t]:s_offs[t] + hi - lo] = St[t, :, lo:hi]
            for t in range(n_xt):
                if gbands[t] is not None:
                    lo, hi = gbands[t]
                    o = s_tot + g_offs[t]
                    sg[:X, o:o + hi - lo] = Gt[t, :, lo:hi]
            in_maps[c][f"xw{s}"] = xw
            sg_parts[c].append(sg)
    order = list(range(1, B_LOC)) + [0]
    for c in range(N_CORES):
        in_maps[c]["sg_a"] = sg_parts[c][order[0]]
        in_maps[c]["sg_b"] = sg_parts[c][order[1]]
        in_maps[c]["sg_rest"] = np.concatenate(
            [sg_parts[c][s] for s in order[2:]], axis=1)
    return tuple(slot_params), in_maps, assign


def _build_nc(slot_params):
    import concourse.bacc as bacc
    import concourse.mybir as mybir
    import concourse.tile as tile

    dt = mybir.dt.float32
    dtd = mybir.dt.float16 if USE_F16 else mybir.dt.float32
    nc = bacc.Bacc(
        "TRN2",
        target_bir_lowering=False,
        debug=False,
        enable_asserts=False,
        num_devices=N_CORES,
    )
    xw_in, sg_meta = [], []
    for s, (n_yt, n_xt, ww, X, sbands, gbands) in enumerate(slot_params):
        s_offs, s_tot = _band_offsets(sbands)
        g_offs, g_tot = _band_offsets(gbands)
        sg_meta.append((s_offs, g_offs, s_tot, g_tot))
        xw_in.append(nc.dram_tensor(f"xw{s}", [128, C, n_yt, ww], dtd,
                                    kind="ExternalInput"))
    out = nc.dram_tensor("out", [B_LOC, 112, 2, C, OUT_W], dtd,
                         kind="ExternalOutput")

    n_slots = len(slot_params)
    # head waits on slot_order[0]'s DMA (keep it small-ish); the tail drain
    # is slot_order[-1]'s pass2 (make it the smallest slot)
    slot_order = list(range(1, n_slots)) + [0]
    # S|G bands ship as three blobs: the first compute slot's piece alone
    # (gates the very first matmul), the second slot's piece, then the rest
    sg_w = [sg_meta[s][2] + sg_meta[s][3] for s in range(n_slots)]
    sg_base = {slot_order[0]: 0, slot_order[1]: 0}
    off = 0
    for s in slot_order[2:]:
        sg_base[s] = off
        off += sg_w[s]
    sga_in = nc.dram_tensor("sg_a", [128, sg_w[slot_order[0]]], dtd,
                            kind="ExternalInput")
    sgb_in = nc.dram_tensor("sg_b", [128, sg_w[slot_order[1]]], dtd,
                            kind="ExternalInput")
    sgr_in = nc.dram_tensor("sg_rest", [128, off], dtd,
                            kind="ExternalInput")
    with tile.TileContext(nc) as tc:
        with (
            tc.tile_pool(name="img", bufs=n_slots) as img_pool,
            tc.tile_pool(name="sg", bufs=3) as sg_pool,
            tc.tile_pool(name="tmp", bufs=3) as tmp_pool,
            tc.tile_pool(name="outp", bufs=3) as out_pool,
            tc.tile_pool(name="ps1a", bufs=3, space="PSUM") as ps1a_pool,
            tc.tile_pool(name="ps1b", bufs=2, space="PSUM") as ps1b_pool,
            tc.tile_pool(name="ps2", bufs=3, space="PSUM") as ps2_pool,
        ):
            tmp_tiles = [None] * n_slots

            # prefetch everything up front: first slot's S|G, its image,
            # then the remaining S|G blob and the other images
            img_tiles = [None] * n_slots

            def img_dma(s):
                n_yt, n_xt, ww, X, _, _ = slot_params[s]
                img_sb = img_pool.tile([128, C, n_yt, ww], dtd)
                nc.sync.dma_start(img_sb[:], xw_in[s][:])
                img_tiles[s] = img_sb

            sga_sb = sg_pool.tile([128, sg_w[slot_order[0]]], dtd,
                                  tag="sga")
            nc.sync.dma_start(sga_sb[:], sga_in[:])
            img_dma(slot_order[0])
            sgb_sb = sg_pool.tile([128, sg_w[slot_order[1]]], dtd,
                                  tag="sgb")
            nc.sync.dma_start(sgb_sb[:], sgb_in[:])
            img_dma(slot_order[1])
            sgr_sb = sg_pool.tile([128, off], dtd, tag="sgr")
            nc.sync.dma_start(sgr_sb[:], sgr_in[:])
            for s in slot_order[2:]:
                img_dma(s)
            sg_tiles = [sgr_sb] * n_slots
            sg_tiles[slot_order[0]] = sga_sb
            sg_tiles[slot_order[1]] = sgb_sb

            def pass1(s):
                n_yt, n_xt, ww, X, sbands, gbands = slot_params[s]
                s_offs, g_offs, s_tot, g_tot = sg_meta[s]
                sg_sb = sg_tiles[s]
                img_sb = img_tiles[s]
                s_emit = [t for t in range(n_yt) if sbands[t] is not None]
                tmp_sb = tmp_pool.tile([128, n_xt, C, OUT_H], dtd)
                tmp_tiles[s] = tmp_sb
                for xb in range(n_xt):
                    xlo = xb * X
                    xn = min(X, ww - xlo)
                    ps1a = ps1a_pool.tile([128, 2 * OUT_H], dt, tag='ps1a')
                    ps1b = ps1b_pool.tile([128, OUT_H], dt, tag='ps1b')
                    # c0+c1 share one accumulation session in ps1a: PSUM
                    # has_written bits make the split ranges exact
                    for c in range(C):
                        base = c * OUT_H if c < 2 else 0
                        ps = ps1a if c < 2 else ps1b
                        for i_t, t in enumerate(s_emit):
                            lo, hi = sbands[t]
                            o = sg_base[s] + s_offs[t]
                            first = i_t == 0 and (c == 0 or c == 2)
                            last = (i_t == len(s_emit) - 1
                                    and (c == 1 or c == 2))
                            nc.tensor.matmul(
                                ps[:xn, base + lo:base + hi],
                                img_sb[:, c, t, xlo:xlo + xn],
                                sg_sb[:, o:o + hi - lo],
                                start=first,
                                stop=last,
                                skip_group_check=True,
                            )
                    # ping-pong the drains so each engine sees half the
                    # burst rate (small slots outrun a single drain engine)
                    if xb % 2 == 0:
                        nc.scalar.copy(tmp_sb[:xn, xb, 0:2, :],
                                       ps1a[:xn, :])
                        nc.vector.tensor_copy(tmp_sb[:xn, xb, 2, :],
                                              ps1b[:xn, :OUT_H])
                    else:
                        nc.vector.tensor_copy(tmp_sb[:xn, xb, 0:2, :],
                                              ps1a[:xn, :])
                        nc.scalar.copy(tmp_sb[:xn, xb, 2, :],
                                       ps1b[:xn, :OUT_H])

            def pass2(s):
                n_yt, n_xt, ww, X, sbands, gbands = slot_params[s]
                s_offs, g_offs, s_tot, g_tot = sg_meta[s]
                sg_sb = sg_tiles[s]
                tmp_sb = tmp_tiles[s]
                g_emit = [t for t in range(n_xt) if gbands[t] is not None]
                out_sb = out_pool.tile([112, 2, C, OUT_W], dtd, tag="out")
                for jb in range(2):
                    ps2a = ps2_pool.tile([112, 2 * OUT_W], dt, tag='ps2')
                    ps2b = ps2_pool.tile([112, OUT_W], dt, tag='ps2')
                    for c in range(C):
                        base = c * OUT_W if c < 2 else 0
                        ps = ps2a if c < 2 else ps2b
                        for i_t, xb in enumerate(g_emit):
                            lo, hi = gbands[xb]
                            o = sg_base[s] + s_tot + g_offs[xb]
                            xn = min(X, ww - xb * X)
                            first = i_t == 0 and (c == 0 or c == 2)
                            last = (i_t == len(g_emit) - 1
                                    and (c == 1 or c == 2))
                            nc.tensor.matmul(
                                ps[:, base + lo:base + hi],
                                tmp_sb[:xn, xb, c,
                                       jb * 112:(jb + 1) * 112],
                                sg_sb[:xn, o:o + hi - lo],
                                start=first,
                                stop=last,
                                skip_group_check=True,
                            )
                    if jb == 0:
                        nc.vector.tensor_copy(out_sb[:, jb, 0:2, :],
                                              ps2a[:, :])
                        nc.scalar.copy(out_sb[:, jb, 2, :],
                                       ps2b[:, :OUT_W])
                    else:
                        nc.scalar.copy(out_sb[:, jb, 0:2, :],
                                       ps2a[:, :])
                        nc.vector.tensor_copy(out_sb[:, jb, 2, :],
                                              ps2b[:, :OUT_W])
                    # per-half output DMA: jb0 ships while jb1 computes,
                    # and the post-compute tail is only half a slot
                    nc.sync.dma_start(out[s][:, jb], out_sb[:, jb])

            # software pipeline: pass2 of the previous slot is emitted after
            # pass1 of the current one, so PE never head-of-line blocks on
            # the ps1->tmp drains.
            prev = None
            for s in slot_order:
                pass1(s)
                if prev is not None:
                    pass2(prev)
                prev = s
            pass2(prev)
    nc.compile()
    return nc


def kernel(x, _trace=False):
    global LAST_EXEC_NS, LAST_RESULTS
    from concourse.bass_utils import run_bass_kernel_spmd

    x = np.ascontiguousarray(np.asarray(x), dtype=np.float32)
    assert x.shape == (B_FULL, C, H, W + 1), x.shape

    slot_params, in_maps, assign = _prepare(x)
    key = (slot_params, USE_F16)
    if key not in _NC_CACHE:
        _NC_CACHE[key] = _build_nc(slot_params)
    nc = _NC_CACHE[key]

    res = run_bass_kernel_spmd(nc, in_maps, list(range(N_CORES)), trace=_trace)
    LAST_EXEC_NS = res.exec_time_ns
    LAST_RESULTS = res

    out_full = np.empty((B_FULL, C, OUT_H, OUT_W), np.float32)
    for s in range(B_LOC):
        for c in range(N_CORES):
            # device layout [112, 2, C, OUT_W]; j = b*112 + p
            arr = res.results[c]["out"][s].astype(np.float32)
            out_full[assign[s][c]] = arr.transpose(2, 1, 0, 3).reshape(
                C, OUT_H, OUT_W)
    return out_full



# revision 15
# speedup vs baseline: 2.2631x; 1.0651x over previous
"""Trainium2 Bass kernel for nn_CenterCrop: per-sample resize(short-side=256)
+ center-crop(224), bilinear, batch sharded over 8 NeuronCores.

Bilinear resize is separable: out = S^T @ img @ G with per-sample sparse
interpolation matrices S (vertical) and G (horizontal), built on the host
from the h/w metadata. The gather+lerp runs on the PE array as matmuls:
  pass1: tmp1_T[x, j] = sum_y img[y, x] * S[y, j]   (img tiles stationary)
  pass2: out[j, i]    = sum_x tmp1_T[x, j] * G[x, i] (tmp1 tiles stationary)

Perf structure (baseline fp32 115.5us -> ~44us measured):
- fp16 data path end-to-end: PE streams at 1 cyc/row (4x the fp32 rate),
  DMA bytes halve, output written fp16 and upcast on host. Bilinear error
  stays ~8e-4 max-rel (gate is 2e-2).
- Only the per-sample source window that the output reads is DMA'd, in a
  partition-major layout so every DMA is one linear run per partition.
- S/G are banded (2 nonzeros per column); only the per-128-row-tile
  nonzero band columns ship, packed into two blobs (first two compute
  slots' bands alone so the first matmul isn't gated on the rest).
- All input DMAs are issued up front (img bufs = n_slots, no reuse WAR).
- PSUM channel pairing: c0+c1 share a [*,448] PSUM tile and a single
  accumulation session (has_written bits make split ranges exact), so
  each x-chunk drains in 2 wide casts instead of 3 narrow ones; the two
  drain engines (Act, DVE) ping-pong per x-chunk/jb so neither sees the
  full burst rate (GpSimd cannot touch PSUM, DMA cannot read it).
- Software pipelining: pass2 of slot s-1 is emitted after pass1 of slot
  s, so the PE queue never head-of-line blocks on the ps1->tmp drains.
- SPMD requires one program for all 8 cores, so samples are sorted by
  min(h,w) and dealt round-robin: slot s on every core holds same-sized
  windows; the program is specialized per-slot to the union shape/bands.
  Outputs are unpermuted/transposed back on the host.
"""

import sys
import os

for _p in ("/opt/trn_rl_repo",):
    if os.path.isdir(_p) and _p not in sys.path:
        sys.path.insert(0, _p)

import numpy as np

OUT_H = 224
OUT_W = 224
RESIZE_TO = np.float32(256.0)
B_FULL = 64
N_CORES = 8
B_LOC = B_FULL // N_CORES  # 8 slots per core
C = 3
H = 512
W = 512  # image width after stripping the metadata column (stored width 513)

LAST_EXEC_NS = None
LAST_RESULTS = None
_NC_CACHE = {}

# fp16 single-pass data path (default). Disable via CENTERCROP_F16=0 for an
# fp32 debugging fallback.
USE_F16 = os.environ.get("CENTERCROP_F16", "1") == "1"


def _interp_matrices(h, w):
    """Full S [512, OUT_H], G [512, OUT_W] fp32 interpolation matrices,
    mirroring the reference fp32 math bit-for-bit."""
    f32 = np.float32
    h = f32(h)
    w = f32(w)
    min_dim = min(h, w)
    scale = RESIZE_TO / min_dim
    h_res = np.round(h * scale)
    w_res = np.round(w * scale)
    top = np.round((h_res - f32(OUT_H)) / f32(2.0))
    left = np.round((w_res - f32(OUT_W)) / f32(2.0))

    def axis_mat(n_out, offset, dim, dim_res, n_src):
        idx = np.arange(n_out, dtype=np.float32) + offset
        src = np.clip((idx + f32(0.5)) * dim / dim_res - f32(0.5),
                      f32(0.0), dim - f32(1.0))
        p0f = np.floor(src)
        frac = src - p0f
        imax = np.int32(dim) - 1
        p0 = np.clip(p0f.astype(np.int32), 0, imax)
        p1 = np.minimum(p0 + 1, imax)
        mat = np.zeros((n_src, n_out), np.float32)
        cols = np.arange(n_out)
        np.add.at(mat, (p0, cols), f32(1.0) - frac)
        np.add.at(mat, (p1, cols), frac)
        return mat

    S = axis_mat(OUT_H, top, h, h_res, H)
    G = axis_mat(OUT_W, left, w, w_res, W)
    return S, G


def _bands(mat_w, n_tiles, tile_rows=128):
    """Per-tile [lo, hi) columns with any nonzero; None if empty."""
    out = []
    for t in range(n_tiles):
        rows = mat_w[t * tile_rows:(t + 1) * tile_rows]
        nz = np.nonzero(rows.any(axis=0))[0]
        out.append(None if nz.size == 0 else (int(nz[0]), int(nz[-1]) + 1))
    return out


def _union_bands(band_lists):
    n = len(band_lists[0])
    out = []
    for t in range(n):
        los = [b[t][0] for b in band_lists if b[t] is not None]
        his = [b[t][1] for b in band_lists if b[t] is not None]
        out.append(None if not los else (min(los), max(his)))
    return out


def _band_offsets(bands):
    """Packed running offsets for non-empty bands; total width last."""
    offs = []
    off = 0
    for b in bands:
        if b is None:
            offs.append(None)
        else:
            offs.append(off)
            off += b[1] - b[0]
    return offs, off


def _prepare(x):
    """Host prep: per-sample windows/matrices, sorted slot assignment,
    per-core packed inputs, and the per-slot program parameters."""
    dtd_np = np.float16 if USE_F16 else np.float32
    h_all = x[:, 0, 0, -1].astype(np.float32)
    w_all = x[:, 1, 0, -1].astype(np.float32)

    samples = []
    for b in range(B_FULL):
        S, G = _interp_matrices(h_all[b], w_all[b])
        ynz = np.nonzero(S.any(axis=1))[0]
        xnz = np.nonzero(G.any(axis=1))[0]
        y0, y1 = int(ynz[0]), int(ynz[-1]) + 1
        x0, x1 = int(xnz[0]), int(xnz[-1]) + 1
        samples.append(dict(S=S[y0:y1], G=G[x0:x1], y0=y0, x0=x0,
                            wh=y1 - y0, ww=x1 - x0))

    order = np.argsort(np.minimum(h_all, w_all), kind="stable")
    # slot s, core c -> sample order[s*N_CORES + c]
    assign = [[int(order[s * N_CORES + c]) for c in range(N_CORES)]
              for s in range(B_LOC)]

    slot_params = []
    slot_data = []  # per slot: list over cores of (sid, Sw_pad, Gw_pad)
    for s in range(B_LOC):
        sids = assign[s]
        wh = max(samples[i]["wh"] for i in sids)
        ww = max(samples[i]["ww"] for i in sids)
        n_yt = (wh + 127) // 128
        n_xt = (ww + 127) // 128
        # balanced x-chunks: the stationary-load (LDWEIGHTS) chain paces
        # the PE at max(xn, band) cols per matmul, so a 128,128,...,thin
        # split wastes LW columns vs even chunks of X = ceil(ww/n_xt).
        X = (ww + n_xt - 1) // n_xt
        sb_list, gb_list, data = [], [], []
        for i in sids:
            sp = samples[i]
            Sw = np.zeros((n_yt * 128, OUT_H), np.float32)
            Sw[:sp["wh"]] = sp["S"]
            Gw = np.zeros((n_xt * X, OUT_W), np.float32)
            Gw[:sp["ww"]] = sp["G"]
            sb_list.append(_bands(Sw, n_yt))
            gb_list.append(_bands(Gw, n_xt, X))
            data.append((i, Sw, Gw))
        sbands = _union_bands(sb_list)
        gbands = _union_bands(gb_list)
        slot_params.append((n_yt, n_xt, ww, X,
                            tuple(sbands), tuple(gbands)))
        slot_data.append(data)

    # pack per-core input maps; all slots' S|G bands share one "sgall"
    in_maps = [{} for _ in range(N_CORES)]
    sg_parts = [[] for _ in range(N_CORES)]
    for s in range(B_LOC):
        n_yt, n_xt, ww, X, sbands, gbands = slot_params[s]
        s_offs, s_tot = _band_offsets(sbands)
        g_offs, g_tot = _band_offsets(gbands)
        for c in range(N_CORES):
            sid, Sw, Gw = slot_data[s][c]
            sp = samples[sid]
            xw = np.zeros((C, n_yt, 128, ww), dtd_np)
            win = x[sid, :, sp["y0"]:sp["y0"] + sp["wh"],
                    sp["x0"]:sp["x0"] + sp["ww"]]
            for t in range(n_yt):
                rows = win[:, t * 128:(t + 1) * 128]
                xw[:, t, :rows.shape[1], :sp["ww"]] = rows
            # partition-major layout so the DMA is one linear run/partition
            xw = np.ascontiguousarray(xw.transpose(2, 0, 1, 3))
            # packed S|G bands: [128, s_tot + g_tot]
            sg = np.zeros((128, s_tot + g_tot), dtd_np)
            St = Sw.reshape(n_yt, 128, OUT_H)
            Gt = Gw.reshape(n_xt, X, OUT_W)
            for t in range(n_yt):
                if sbands[t] is not None:
                    lo, hi = sbands[t]
                    sg[:, s_offs[t]:s_offs[t] + hi - lo] = St[t, :, lo:hi]
            for t in range(n_xt):
                if gbands[t] is not None:
                    lo, hi = gbands[t]
                    o = s_tot + g_offs[t]
                    sg[:X, o:o + hi - lo] = Gt[t, :, lo:hi]
            in_maps[c][f"xw{s}"] = xw
            sg_parts[c].append(sg)
    order = list(range(1, B_LOC)) + [0]
    for c in range(N_CORES):
        in_maps[c]["sg_a"] = sg_parts[c][order[0]]
        in_maps[c]["sg_b"] = sg_parts[c][order[1]]
        in_maps[c]["sg_rest"] = np.concatenate(
            [sg_parts[c][s] for s in order[2:]], axis=1)
    return tuple(slot_params), in_maps, assign


def _build_nc(slot_params):
    import concourse.bacc as bacc
    import concourse.mybir as mybir
    import concourse.tile as tile

    dt = mybir.dt.float32
    dtd = mybir.dt.float16 if USE_F16 else mybir.dt.float32
    nc = bacc.Bacc(
        "TRN2",
        target_bir_lowering=False,
        debug=False,
        enable_asserts=False,
        num_devices=N_CORES,
    )
    xw_in, sg_meta = [], []
    for s, (n_yt, n_xt, ww, X, sbands, gbands) in enumerate(slot_params):
        s_offs, s_tot = _band_offsets(sbands)
        g_offs, g_tot = _band_offsets(gbands)
        sg_meta.append((s_offs, g_offs, s_tot, g_tot))
        xw_in.append(nc.dram_tensor(f"xw{s}", [128, C, n_yt, ww], dtd,
                                    kind="ExternalInput"))
    out = nc.dram_tensor("out", [B_LOC, 112, 2, C, OUT_W], dtd,
                         kind="ExternalOutput")

    n_slots = len(slot_params)
    # head waits on slot_order[0]'s DMA (keep it small-ish); the tail drain
    # is slot_order[-1]'s pass2 (make it the smallest slot)
    slot_order = list(range(1, n_slots)) + [0]
    # S|G bands ship as three blobs: the first compute slot's piece alone
    # (gates the very first matmul), the second slot's piece, then the rest
    sg_w = [sg_meta[s][2] + sg_meta[s][3] for s in range(n_slots)]
    sg_base = {slot_order[0]: 0, slot_order[1]: 0}
    off = 0
    for s in slot_order[2:]:
        sg_base[s] = off
        off += sg_w[s]
    sga_in = nc.dram_tensor("sg_a", [128, sg_w[slot_order[0]]], dtd,
                            kind="ExternalInput")
    sgb_in = nc.dram_tensor("sg_b", [128, sg_w[slot_order[1]]], dtd,
                            kind="ExternalInput")
    sgr_in = nc.dram_tensor("sg_rest", [128, off], dtd,
                            kind="ExternalInput")
    with tile.TileContext(nc) as tc:
        with (
            tc.tile_pool(name="img", bufs=n_slots) as img_pool,
            tc.tile_pool(name="sg", bufs=3) as sg_pool,
            tc.tile_pool(name="tmp", bufs=3) as tmp_pool,
            tc.tile_pool(name="outp", bufs=3) as out_pool,
            tc.tile_pool(name="ps1a", bufs=3, space="PSUM") as ps1a_pool,
            tc.tile_pool(name="ps1b", bufs=2, space="PSUM") as ps1b_pool,
            tc.tile_pool(name="ps2", bufs=3, space="PSUM") as ps2_pool,
        ):
            tmp_tiles = [None] * n_slots

            # prefetch everything up front: first slot's S|G, its image,
            # then the remaining S|G blob and the other images
            img_tiles = [None] * n_slots

            def img_dma(s):
                n_yt, n_xt, ww, X, _, _ = slot_params[s]
                img_sb = img_pool.tile([128, C, n_yt, ww], dtd)
                nc.sync.dma_start(img_sb[:], xw_in[s][:])
                img_tiles[s] = img_sb

            sga_sb = sg_pool.tile([128, sg_w[slot_order[0]]], dtd,
                                  tag="sga")
            nc.sync.dma_start(sga_sb[:], sga_in[:])
            img_dma(slot_order[0])
            sgb_sb = sg_pool.tile([128, sg_w[slot_order[1]]], dtd,
                                  tag="sgb")
            nc.sync.dma_start(sgb_sb[:], sgb_in[:])
            img_dma(slot_order[1])
            sgr_sb = sg_pool.tile([128, off], dtd, tag="sgr")
            nc.sync.dma_start(sgr_sb[:], sgr_in[:])
            # hoist the biggest remaining image to the front of the rest:
            # it is reached while the stream is still draining, and a late
            # arrival there stalls the PE long enough to drop the clock
            # boost; the others keep compute order
            rest = list(slot_order[2:])
            big = max(rest, key=lambda s: slot_params[s][2])
            rest.remove(big)
            for s in [big] + rest:
                img_dma(s)
            sg_tiles = [sgr_sb] * n_slots
            sg_tiles[slot_order[0]] = sga_sb
            sg_tiles[slot_order[1]] = sgb_sb

            def pass1(s):
                n_yt, n_xt, ww, X, sbands, gbands = slot_params[s]
                s_offs, g_offs, s_tot, g_tot = sg_meta[s]
                sg_sb = sg_tiles[s]
                img_sb = img_tiles[s]
                s_emit = [t for t in range(n_yt) if sbands[t] is not None]
                tmp_sb = tmp_pool.tile([128, n_xt, C, OUT_H], dtd)
                tmp_tiles[s] = tmp_sb
                for xb in range(n_xt):
                    xlo = xb * X
                    xn = min(X, ww - xlo)
                    ps1a = ps1a_pool.tile([128, 2 * OUT_H], dt, tag='ps1a')
                    ps1b = ps1b_pool.tile([128, OUT_H], dt, tag='ps1b')
                    # c0+c1 share one accumulation session in ps1a: PSUM
                    # has_written bits make the split ranges exact
                    for c in range(C):
                        base = c * OUT_H if c < 2 else 0
                        ps = ps1a if c < 2 else ps1b
                        for i_t, t in enumerate(s_emit):
                            lo, hi = sbands[t]
                            o = sg_base[s] + s_offs[t]
                            first = i_t == 0 and (c == 0 or c == 2)
                            last = (i_t == len(s_emit) - 1
                                    and (c == 1 or c == 2))
                            nc.tensor.matmul(
                                ps[:xn, base + lo:base + hi],
                                img_sb[:, c, t, xlo:xlo + xn],
                                sg_sb[:, o:o + hi - lo],
                                start=first,
                                stop=last,
                                skip_group_check=True,
                            )
                    # ping-pong the drains so each engine sees half the
                    # burst rate (small slots outrun a single drain engine)
                    if xb % 2 == 0:
                        nc.scalar.copy(tmp_sb[:xn, xb, 0:2, :],
                                       ps1a[:xn, :])
                        nc.vector.tensor_copy(tmp_sb[:xn, xb, 2, :],
                                              ps1b[:xn, :OUT_H])
                    else:
                        nc.vector.tensor_copy(tmp_sb[:xn, xb, 0:2, :],
                                              ps1a[:xn, :])
                        nc.scalar.copy(tmp_sb[:xn, xb, 2, :],
                                       ps1b[:xn, :OUT_H])

            def pass2(s):
                n_yt, n_xt, ww, X, sbands, gbands = slot_params[s]
                s_offs, g_offs, s_tot, g_tot = sg_meta[s]
                sg_sb = sg_tiles[s]
                tmp_sb = tmp_tiles[s]
                g_emit = [t for t in range(n_xt) if gbands[t] is not None]
                out_sb = out_pool.tile([112, 2, C, OUT_W], dtd, tag="out")
                for jb in range(2):
                    ps2a = ps2_pool.tile([112, 2 * OUT_W], dt, tag='ps2')
                    ps2b = ps2_pool.tile([112, OUT_W], dt, tag='ps2')
                    for c in range(C):
                        base = c * OUT_W if c < 2 else 0
                        ps = ps2a if c < 2 else ps2b
                        for i_t, xb in enumerate(g_emit):
                            lo, hi = gbands[xb]
                            o = sg_base[s] + s_tot + g_offs[xb]
                            xn = min(X, ww - xb * X)
                            first = i_t == 0 and (c == 0 or c == 2)
                            last = (i_t == len(g_emit) - 1
                                    and (c == 1 or c == 2))
                            nc.tensor.matmul(
                                ps[:, base + lo:base + hi],
                                tmp_sb[:xn, xb, c,
                                       jb * 112:(jb + 1) * 112],
                                sg_sb[:xn, o:o + hi - lo],
                                start=first,
                                stop=last,
                                skip_group_check=True,
                            )
                    if jb == 0:
                        nc.vector.tensor_copy(out_sb[:, jb, 0:2, :],
                                              ps2a[:, :])
                        nc.scalar.copy(out_sb[:, jb, 2, :],
                                       ps2b[:, :OUT_W])
                    else:
                        nc.scalar.copy(out_sb[:, jb, 0:2, :],
                                       ps2a[:, :])
                        nc.vector.tensor_copy(out_sb[:, jb, 2, :],
                                              ps2b[:, :OUT_W])
                    # per-half output DMA: jb0 ships while jb1 computes,
                    # and the post-compute tail is only half a slot
                    nc.sync.dma_start(out[s][:, jb], out_sb[:, jb])

            # software pipeline: pass2 of the previous slot is emitted after
            # pass1 of the current one, so PE never head-of-line blocks on
            # the ps1->tmp drains.
            prev = None
            for s in slot_order:
                pass1(s)
                if prev is not None:
                    pass2(prev)
                prev = s
            pass2(prev)
    nc.compile()
    return nc


def kernel(x, _trace=False):
    global LAST_EXEC_NS, LAST_RESULTS
    from concourse.bass_utils import run_bass_kernel_spmd

    x = np.ascontiguousarray(np.asarray(x), dtype=np.float32)
    assert x.shape == (B_FULL, C, H, W + 1), x.shape

    slot_params, in_maps, assign = _prepare(x)
    key = (slot_params, USE_F16)
    if key not in _NC_CACHE:
        _NC_CACHE[key] = _build_nc(slot_params)
    nc = _NC_CACHE[key]

    res = run_bass_kernel_spmd(nc, in_maps, list(range(N_CORES)), trace=_trace)
    LAST_EXEC_NS = res.exec_time_ns
    LAST_RESULTS = res

    out_full = np.empty((B_FULL, C, OUT_H, OUT_W), np.float32)
    for s in range(B_LOC):
        for c in range(N_CORES):
            # device layout [112, 2, C, OUT_W]; j = b*112 + p
            arr = res.results[c]["out"][s].astype(np.float32)
            out_full[assign[s][c]] = arr.transpose(2, 1, 0, 3).reshape(
                C, OUT_H, OUT_W)
    return out_full

